# revision 26
# baseline (speedup 1.0000x reference)
"""Trainium2 Bass kernel for 5x5 median filter (reflect padding, SAME size).

Input x: [16, 384, 384, 3] f32 (NHWC), k=5. Output: same shape.

Strategy:
- Pure data parallel over 8 NeuronCores: 2 images per core.
- All compute in bf16: DVE tensor_tensor runs in 2x_1P perf mode for
  16-bit dtypes (vs 1x for f32). Median selection commutes with the
  monotone f32->bf16 rounding, so the result equals round_bf16(true
  median): rel err <= 2^-9. Host converts f32<->bf16 at the edges.
- Per core layout: partition p = img*64 + hblock, each hblock = 6 output
  rows. Free dim = (10 input rows) x (100 px * 3 ch) for a 96-px chunk
  (2 px halo each side). 4 chunks cover W=384.
- 2x_1P needs 4B-aligned operands; a 1-px shift is 3 bf16 els = 6B.
  All DVE ops use even element offsets {0, 6}; odd shifts are
  materialized as shifted copies (s' = s<<1px, pm' = pm<<1px) on the
  otherwise-idle Scalar engine, ordered so they hide under DVE work.
- Median-of-25 via separable sorting network, emitted as FUSED
  multi-plane DVE instructions (same-ALU ops over several planes in
  one instruction via slot-major tiles and strided 4-dim APs) to
  amortize the ~150-cycle per-instruction overhead:
  1. vertical sort of 5-row columns (shared across horizontal windows)
  2. PM[x] = Batcher merge of sorted columns (x, x+1) -> sorted 10
  3. per window: L=PM[w-2], R=PM[w+1], M=sorted col w;
     u = mid-6 of merge(L,R) via DCE'd odd/even partial merges (the
     o/e partials are structurally identical -> fused 2-slot ops);
     median = rank 6 of merge(u, M).
- Reflect padding: row halos via DMAs from reflected rows, column halos
  via on-chip copies at image edges. DMAs round-robin over engine
  queues so chunk-0 issue latency shrinks.
"""

import numpy as np
import ml_dtypes

import concourse.bacc as bacc
import concourse.bass as bass
import concourse.mybir as mybir
from concourse.bass_utils import run_bass_kernel_spmd
from concourse.tile import TileContext

bf16 = mybir.dt.bfloat16
AMIN = mybir.AluOpType.min
AMAX = mybir.AluOpType.max

H = 384
W = 384
C = 3
ROW = W * C          # 1152 elements per image row
IMG = H * ROW        # elements per image
R = 6                # output rows per partition block
NBLK = H // R        # 64 blocks per image
W_CHUNK = 96         # output px per chunk
N_CHUNK = W // W_CHUNK

WS = (W_CHUNK + 4) * C    # column-sort domain width (els) = 300
WPM = 298                 # pair-merge op width (even, padded from 297)
WSEL = W_CHUNK * C        # selection/output domain width = 288
TW = 300                  # physical tile width for all work planes


# ---------------------------------------------------------------------------
# Emission context: bass backend + numpy simulation backend (for testing
# the op list without hardware).
# ---------------------------------------------------------------------------

class BassCtx:
    def __init__(self, nc, wp):
        self.nc = nc
        self.wp = wp
        self._tiles = {}

    def tile(self, tag, nslots):
        t = self.wp.tile([128, nslots * R, TW], bf16, tag=tag, name=tag)
        self._tiles[tag] = t
        return t

    # views ------------------------------------------------------------
    def sl(self, t, s0, ns=1, off=0, w=TW):
        """Contiguous slot range [s0, s0+ns), column window [off, off+w)."""
        return t[:, s0 * R:(s0 + ns) * R, off:off + w]

    def st(self, t, sl_, off=0, w=TW):
        """Strided slot view: sl_ is a python slice over slots."""
        r = t.rearrange("p (s r) w -> p s r w", r=R)
        return r[:, sl_, :, off:off + w]

    def xt_rows(self, xt, r0, nr, off=0, w=TW):
        return xt[:, r0:r0 + nr, off:off + w]

    def xt_l1(self, xt, r0, w=TW):
        """Rows {r0..r0+5} and {r0+3..r0+8} as a fused 2-slot view."""
        return bass.AP(xt.tensor, r0 * TW,
                       [[10 * TW, 128], [3 * TW, 2], [TW, R], [1, w]])

    # ops --------------------------------------------------------------
    def tt(self, op, out, in0, in1):
        self.nc.vector.tensor_tensor(out=out, in0=in0, in1=in1, op=op)

    def scopy(self, out, in_):
        self.nc.scalar.copy(out=out, in_=in_)

    def gcopy(self, out, in_):
        self.nc.gpsimd.tensor_copy(out, in_)

    def vcopy(self, out, in_):
        self.nc.vector.tensor_copy(out, in_)


class NumpyCtx:
    """Same op vocabulary over numpy arrays shaped [128, rows, TW]."""

    def __init__(self):
        self._tiles = {}

    def tile(self, tag, nslots):
        a = np.full((128, nslots * R, TW), np.nan, dtype=np.float32)
        self._tiles[tag] = a
        return a

    def sl(self, t, s0, ns=1, off=0, w=TW):
        return t[:, s0 * R:(s0 + ns) * R, off:off + w]

    def st(self, t, sl_, off=0, w=TW):
        r = t.reshape(128, -1, R, TW)
        return r[:, sl_, :, off:off + w]

    def xt_rows(self, xt, r0, nr, off=0, w=TW):
        return xt[:, r0:r0 + nr, off:off + w]

    def xt_l1(self, xt, r0, w=TW):
        v = np.lib.stride_tricks.as_strided(
            xt[:, r0:, :], shape=(128, 2, R, w),
            strides=(xt.strides[0], 3 * xt.strides[1], xt.strides[1],
                     xt.strides[2]))
        return v

    def tt(self, op, out, in0, in1):
        f = np.minimum if op is AMIN else np.maximum
        res = f(in0.reshape(out.shape), in1.reshape(out.shape))
        out[...] = res

    def scopy(self, out, in_):
        out[...] = in_.reshape(out.shape)

    gcopy = scopy
    vcopy = scopy


# ---------------------------------------------------------------------------
# One chunk: sort -> merge -> selection, with fused DVE ops and
# ScalarE shifted copies.
# ---------------------------------------------------------------------------

def emit_l1(ctx, xt, tiles):
    """First sort layer: CE(0,1), CE(3,4), fused. Emitted one chunk
    ahead so it fills the DVE stall while ScalarE finishes the s'
    copies of the previous chunk."""
    X2, X3 = tiles["X2"], tiles["X3"]
    ctx.tt(AMIN, ctx.sl(X2, 0, 2), ctx.xt_l1(xt, 0), ctx.xt_l1(xt, 1))
    ctx.tt(AMAX, ctx.sl(X3, 0, 2), ctx.xt_l1(xt, 0), ctx.xt_l1(xt, 1))


def emit_sort_rest(ctx, xt, tiles):
    S, SP, LH, Q, FP, X2, X3 = (tiles[k] for k in
                                ("S", "SP", "LH", "Q", "FP", "X2", "X3"))
    MIN, MAX = AMIN, AMAX
    sl = ctx.sl
    # X2 = (v0, v3), X3 = (v1, v4) from emit_l1.
    ctx.tt(MIN, sl(FP, 0), ctx.xt_rows(xt, 2, R), sl(X3, 1))       # (2,4)lo
    ctx.tt(MAX, sl(FP, 1), ctx.xt_rows(xt, 2, R), sl(X3, 1))       # (2,4)hi
    ctx.tt(MIN, sl(FP, 2), sl(FP, 0), sl(X2, 1))                   # (2,3)lo
    ctx.tt(MAX, sl(FP, 3), sl(FP, 0), sl(X2, 1))                   # (2,3)hi
    ctx.tt(MIN, sl(LH, 0), sl(X3, 0), sl(FP, 1))                   # (1,4)lo
    ctx.tt(MAX, sl(S, 4), sl(X3, 0), sl(FP, 1))                    # s4
    ctx.scopy(sl(SP, 4, 1, 0, WS - C), sl(S, 4, 1, C, WS - C))
    ctx.tt(MIN, sl(LH, 1), sl(X2, 0), sl(FP, 3))                   # (0,3)lo
    ctx.tt(MAX, sl(LH, 2), sl(X2, 0), sl(FP, 3))                   # (0,3)hi
    ctx.tt(MIN, sl(S, 0), sl(LH, 1), sl(FP, 2))                    # s0
    ctx.tt(MAX, sl(LH, 3), sl(LH, 1), sl(FP, 2))                   # (0,2)hi
    ctx.scopy(sl(SP, 0, 1, 0, WS - C), sl(S, 0, 1, C, WS - C))
    ctx.tt(MIN, sl(LH, 4), sl(LH, 0), sl(LH, 2))                   # (1,3)lo
    ctx.tt(MAX, sl(S, 3), sl(LH, 0), sl(LH, 2))                    # s3
    ctx.scopy(sl(SP, 3, 1, 0, WS - C), sl(S, 3, 1, C, WS - C))
    ctx.tt(MIN, sl(S, 1), sl(LH, 4), sl(LH, 3))                    # s1
    ctx.tt(MAX, sl(S, 2), sl(LH, 4), sl(LH, 3))                    # s2
    ctx.scopy(sl(SP, 1, 1, 0, WS - C), sl(S, 1, 1, C, WS - C))
    # s2's copy is on the critical path (last sort output): the DVE's
    # own 2x_2P tensor_copy (~1us) is cheaper than stalling on ScalarE
    ctx.vcopy(sl(SP, 2, 1, 0, WS - C), sl(S, 2, 1, C, WS - C))


def emit_merge_sel(ctx, outt_halves, tiles):
    S, SP, LH, PM, PMP, Q, FP = (tiles[k] for k in
                                 ("S", "SP", "LH", "PM", "PMP", "Q", "FP"))
    MIN, MAX = AMIN, AMAX
    sl, st = ctx.sl, ctx.st

    # ---- pair merge: PM[x] = merge(s[x], s[x+1]), b = s' ----
    # LH slots 0..4 = lo_i = min(a_i, b_i); slots 5..9 = hi_i.
    ctx.tt(MIN, sl(LH, 0, 5, 0, WPM), sl(S, 0, 5, 0, WPM),
           sl(SP, 0, 5, 0, WPM))
    ctx.tt(MAX, sl(LH, 5, 5, 0, WPM), sl(S, 0, 5, 0, WPM),
           sl(SP, 0, 5, 0, WPM))
    # pm0 = lo0, pm9 = hi4 (+ their shifted copies) on ScalarE
    ctx.scopy(sl(PM, 0, 1, 0, WPM), sl(LH, 0, 1, 0, WPM))
    ctx.scopy(sl(PMP, 0, 1, 0, WPM - C), sl(LH, 0, 1, C, WPM - C))
    ctx.scopy(sl(PM, 9, 1, 0, WPM), sl(LH, 9, 1, 0, WPM))
    ctx.scopy(sl(PMP, 9, 1, 0, WPM - C), sl(LH, 9, 1, C, WPM - C))
    # Q = (h1m, gm1, h2m, gm2): merges of (hi0,lo4) and (hi1,lo3)
    ctx.tt(MIN, sl(Q, 0, 2, 0, WPM), st(LH, slice(5, 7), 0, WPM),
           st(LH, slice(4, 2, -1), 0, WPM))
    ctx.tt(MAX, sl(Q, 2, 2, 0, WPM), st(LH, slice(5, 7), 0, WPM),
           st(LH, slice(4, 2, -1), 0, WPM))
    # FP = (f1, f2, f3, f4): f1,f2 = merge(lo2, h1m); f3,f4 = (hi2, h2m)
    ctx.tt(MIN, st(FP, slice(0, 3, 2), 0, WPM),
           st(LH, slice(2, 8, 5), 0, WPM), st(Q, slice(0, 3, 2), 0, WPM))
    ctx.tt(MAX, st(FP, slice(1, 4, 2), 0, WPM),
           st(LH, slice(2, 8, 5), 0, WPM), st(Q, slice(0, 3, 2), 0, WPM))
    # loop: pm_{2i+1} = min(g_i, f_{i+1}); pm_{2i+2} = max.
    # g = (lo1, gm1, gm2, hi3) = (LH1, Q1, Q3, LH8); f = FP0..FP3.
    # singles, ordered so the ScalarE pm' copies (same order) finish
    # before the selection needs them: {2,3} first, then {6,7},{4,5},
    # {0,1},{8,9} pairs.
    g_ops = [  # (pm_idx, op, g_view, f_view)
        (2, MAX, sl(LH, 1, 1, 0, WPM), sl(FP, 0, 1, 0, WPM)),
        (3, MIN, sl(Q, 1, 1, 0, WPM), sl(FP, 1, 1, 0, WPM)),
        (6, MAX, sl(Q, 3, 1, 0, WPM), sl(FP, 2, 1, 0, WPM)),
        (7, MIN, sl(LH, 8, 1, 0, WPM), sl(FP, 3, 1, 0, WPM)),
        (4, MAX, sl(Q, 1, 1, 0, WPM), sl(FP, 1, 1, 0, WPM)),
        (5, MIN, sl(Q, 3, 1, 0, WPM), sl(FP, 2, 1, 0, WPM)),
        (1, MIN, sl(LH, 1, 1, 0, WPM), sl(FP, 0, 1, 0, WPM)),
        (8, MAX, sl(LH, 8, 1, 0, WPM), sl(FP, 3, 1, 0, WPM)),
    ]
    for pi, op, gv, fv in g_ops:
        ctx.tt(op, sl(PM, pi, 1, 0, WPM), gv, fv)
        ctx.scopy(sl(PMP, pi, 1, 0, WPM - C), sl(PM, pi, 1, C, WPM - C))

    # ---- selection: mid-6 of merge(L,R), then rank 6 of merge(u, M) --
    # Fused 2-slot ops; sub-slot 0 = e-variant (even pm), 1 = o-variant.
    # Operand pairs: t1:{2,3} t2:{6,7} k:{4,5} t3:{0,1} t4:{8,9}.
    def pmpair(i):
        return (sl(PM, 2 * i, 2, 0, WSEL), sl(PMP, 2 * i, 2, 2 * C, WSEL))

    W2 = WSEL
    t1, t1b = sl(SP, 0, 2, 0, W2), pmpair(1)
    ctx.tt(MAX, t1, *t1b)
    t2 = sl(SP, 2, 2, 0, W2)
    ctx.tt(MIN, t2, *pmpair(3))
    g1 = sl(LH, 0, 2, 0, W2)
    ctx.tt(MIN, g1, t1, t2)
    g2 = sl(LH, 2, 2, 0, W2)
    ctx.tt(MAX, g2, t1, t2)
    k0 = sl(LH, 5, 2, 0, W2)
    ctx.tt(MIN, k0, *pmpair(2))
    k1 = sl(LH, 7, 2, 0, W2)
    ctx.tt(MAX, k1, *pmpair(2))
    t3 = sl(Q, 0, 2, 0, W2)
    ctx.tt(MAX, t3, *pmpair(0))
    t4 = sl(Q, 2, 2, 0, W2)
    ctx.tt(MIN, t4, *pmpair(4))
    h1 = sl(FP, 0, 2, 0, W2)
    ctx.tt(MIN, h1, t3, t4)
    h2 = sl(FP, 2, 2, 0, W2)
    ctx.tt(MAX, h2, t3, t4)
    f2 = sl(SP, 0, 2, 0, W2)      # overwrites t1 (dead)
    ctx.tt(MAX, f2, k0, h1)
    f3 = sl(SP, 2, 2, 0, W2)      # overwrites t2 (dead)
    ctx.tt(MIN, f3, k1, h2)

    # singles: o3,o4,o5 -> LH slots 5,6,7 ; e4,e5,e6 -> Q slots 0,1,2
    def sub(v2, j):  # sub-slot j of a 2-slot view
        return v2[:, j * R:(j + 1) * R, :]

    ctx.tt(MIN, sl(LH, 5, 1, 0, W2), sub(g1, 1), sub(f2, 1))   # o3
    ctx.tt(MAX, sl(LH, 6, 1, 0, W2), sub(g1, 1), sub(f2, 1))   # o4
    ctx.tt(MIN, sl(LH, 7, 1, 0, W2), sub(g2, 1), sub(f3, 1))   # o5
    ctx.tt(MAX, sl(Q, 0, 1, 0, W2), sub(g1, 0), sub(f2, 0))    # e4
    ctx.tt(MIN, sl(Q, 1, 1, 0, W2), sub(g2, 0), sub(f3, 0))    # e5
    ctx.tt(MAX, sl(Q, 2, 1, 0, W2), sub(g2, 0), sub(f3, 0))    # e6

    # u_i -> PM slots 0..5 (dead by now): u0,u2,u4 = min(o,e) strided
    ctx.tt(MIN, st(PM, slice(0, 6, 2), 0, W2), sl(LH, 5, 3, 0, W2),
           sl(Q, 0, 3, 0, W2))
    ctx.tt(MAX, st(PM, slice(1, 6, 2), 0, W2), sl(LH, 5, 3, 0, W2),
           sl(Q, 0, 3, 0, W2))

    # final: fused (x2,x1,k1p) = max(u_i, M_i) i=0..2 ;
    #        (q0,x3) = min(u_i, M_i) i=3..4
    ctx.tt(MAX, sl(FP, 0, 3, 0, W2), sl(PM, 0, 3, 0, W2),
           sl(S, 0, 3, 2 * C, W2))
    ctx.tt(MIN, sl(Q, 0, 2, 0, W2), sl(PM, 3, 2, 0, W2),
           sl(S, 3, 2, 2 * C, W2))
    ctx.tt(MIN, sl(LH, 0, 1, 0, W2), sl(FP, 1, 1, 0, W2),
           sl(PM, 5, 1, 0, W2))                                 # p1
    ctx.tt(MAX, sl(LH, 1, 1, 0, W2), sl(Q, 0, 1, 0, W2),
           sl(LH, 0, 1, 0, W2))                                 # o2p
    ctx.tt(MAX, sl(LH, 2, 1, 0, W2), sl(FP, 0, 1, 0, W2),
           sl(Q, 1, 1, 0, W2))                                  # h2p
    ctx.tt(MIN, sl(LH, 3, 1, 0, W2), sl(FP, 2, 1, 0, W2),
           sl(LH, 2, 1, 0, W2))                                 # e3p
    outv, post = outt_halves
    ctx.tt(MIN, outv, sl(LH, 1, 1, 0, W2), sl(LH, 3, 1, 0, W2))  # median
    post()


# ---------------------------------------------------------------------------
# Kernel builder
# ---------------------------------------------------------------------------

def build_nc():
    nc = bacc.Bacc("TRN2", target_bir_lowering=False)
    x = nc.dram_tensor("x", [2, H, W, C], bf16, kind="ExternalInput")
    y = nc.dram_tensor("out", [2, H, W, C], bf16, kind="ExternalOutput")
    dma_engs0 = [nc.sync, nc.gpsimd, nc.scalar]
    dma_engs = [nc.sync, nc.gpsimd]

    with TileContext(nc) as tc:
        with tc.tile_pool(name="io", bufs=2) as iop, \
             tc.tile_pool(name="work", bufs=1) as wp:
            ctx = BassCtx(nc, wp)
            tiles = {k: ctx.tile(k, n) for k, n in
                     (("S", 5), ("SP", 5), ("LH", 10), ("PM", 10),
                      ("PMP", 10), ("Q", 4), ("FP", 4), ("X2", 2),
                      ("X3", 2))}
            qi = 0

            def dma(out, in_, engs):
                nonlocal qi
                engs[qi % len(engs)].dma_start(out=out, in_=in_)
                qi += 1

            def emit_load(ci):
                w0 = ci * W_CHUNK
                pxlo = max(0, w0 - 2)
                pxhi = min(W, w0 + W_CHUNK + 2)
                n = (pxhi - pxlo) * C
                elo = (pxlo - (w0 - 2)) * C
                engs = dma_engs0 if ci == 0 else dma_engs
                xt = iop.tile([128, 10, WS], bf16, tag="xt", name="xt")
                for img in range(2):
                    base = img * IMG + pxlo * C
                    p0 = img * NBLK
                    # chunk 0 is latency-critical (nothing to overlap):
                    # split finer so transfers parallelize across rings
                    splits = ([1, 63]
                              if ci == 0 else [1, 17, 33, 48, 63])
                    for si in range(len(splits) - 1):
                        h0, h1 = splits[si], splits[si + 1]
                        src = bass.AP(x, base + (6 * h0 - 2) * ROW,
                                      [[6 * ROW, h1 - h0], [ROW, 10], [1, n]])
                        dma(xt[p0 + h0:p0 + h1, :, elo:elo + n], src, engs)
                    src = bass.AP(x, base, [[ROW, 1], [ROW, 8], [1, n]])
                    dma(xt[p0:p0 + 1, 2:10, elo:elo + n], src, engs)
                    for j, r in ((0, 2), (1, 1)):
                        src = bass.AP(x, base + r * ROW, [[ROW, 1], [1, n]])
                        dma(xt[p0:p0 + 1, j:j + 1, elo:elo + n], src, engs)
                    p63 = p0 + NBLK - 1
                    src = bass.AP(x, base + 376 * ROW,
                                  [[ROW, 1], [ROW, 8], [1, n]])
                    dma(xt[p63:p63 + 1, 0:8, elo:elo + n], src, engs)
                    for j, r in ((8, 382), (9, 381)):
                        src = bass.AP(x, base + r * ROW, [[ROW, 1], [1, n]])
                        dma(xt[p63:p63 + 1, j:j + 1, elo:elo + n], src, engs)

                if ci == 0:
                    nc.scalar.copy(out=xt[:, :, 0:C],
                                   in_=xt[:, :, 4 * C:5 * C])
                    nc.scalar.copy(out=xt[:, :, C:2 * C],
                                   in_=xt[:, :, 3 * C:4 * C])
                if ci == N_CHUNK - 1:
                    wc = W_CHUNK
                    nc.scalar.copy(out=xt[:, :, (wc + 2) * C:(wc + 3) * C],
                                   in_=xt[:, :, wc * C:(wc + 1) * C])
                    nc.scalar.copy(out=xt[:, :, (wc + 3) * C:(wc + 4) * C],
                                   in_=xt[:, :, (wc - 1) * C:wc * C])
                return xt

            xt = emit_load(0)
            emit_l1(ctx, xt, tiles)
            for ci in range(N_CHUNK):
                w0 = ci * W_CHUNK
                emit_sort_rest(ctx, xt, tiles)
                if ci + 1 < N_CHUNK:
                    xt = emit_load(ci + 1)
                    emit_l1(ctx, xt, tiles)

                outt = iop.tile([128, R, WSEL], bf16, tag="outt", name="outt",
                                bufs=1)

                def post(outt=outt, w0=w0):
                    oengs = [nc.sync, nc.gpsimd]
                    for img in range(2):
                        p0 = img * NBLK
                        half = NBLK // 2
                        for hs in range(2):
                            dst = bass.AP(
                                y, img * IMG + hs * half * R * ROW + w0 * C,
                                [[R * ROW, half], [ROW, R], [1, WSEL]])
                            dma(dst,
                                outt[p0 + hs * half:p0 + (hs + 1) * half,
                                     :, :], oengs)

                emit_merge_sel(ctx, (outt[:], post), tiles)

    nc.finalize()
    return nc


# ---------------------------------------------------------------------------
# Numpy simulation of one core (for offline verification of the op list)
# ---------------------------------------------------------------------------

def simulate_core(x2):
    """x2: [2, H, W, C] float32 (pre-rounded to bf16 grid). Returns
    [2, H, W, C] median-filter output computed via the exact op list."""
    out = np.zeros_like(x2)
    xp = np.pad(x2, ((0, 0), (2, 2), (0, 0), (0, 0)), mode="reflect")
    for ci in range(N_CHUNK):
        w0 = ci * W_CHUNK
        pxlo = max(0, w0 - 2)
        pxhi = min(W, w0 + W_CHUNK + 2)
        n = (pxhi - pxlo) * C
        elo = (pxlo - (w0 - 2)) * C

        ctx = NumpyCtx()
        tiles = {k: ctx.tile(k, nsl) for k, nsl in
                 (("S", 5), ("SP", 5), ("LH", 10), ("PM", 10),
                  ("PMP", 10), ("Q", 4), ("FP", 4), ("X2", 2), ("X3", 2))}
        xt = np.full((128, 10, WS), np.nan, dtype=np.float32)
        for img in range(2):
            p0 = img * NBLK
            rows = xp[img].reshape(H + 4, ROW)
            for hb in range(NBLK):
                r0 = hb * R  # padded-row index of first input row
                xt[p0 + hb, :, elo:elo + n] = \
                    rows[r0:r0 + 10, pxlo * C:pxlo * C + n]
        if ci == 0:
            xt[:, :, 0:C] = xt[:, :, 4 * C:5 * C]
            xt[:, :, C:2 * C] = xt[:, :, 3 * C:4 * C]
        if ci == N_CHUNK - 1:
            wc = W_CHUNK
            xt[:, :, (wc + 2) * C:(wc + 3) * C] = xt[:, :, wc * C:(wc + 1) * C]
            xt[:, :, (wc + 3) * C:(wc + 4) * C] = \
                xt[:, :, (wc - 1) * C:wc * C]

        outt = np.full((128, R, WSEL), np.nan, dtype=np.float32)
        emit_l1(ctx, xt, tiles)
        emit_sort_rest(ctx, xt, tiles)
        emit_merge_sel(ctx, (outt, lambda: None), tiles)

        for img in range(2):
            p0 = img * NBLK
            o = outt[p0:p0 + NBLK].reshape(H, WSEL)
            out[img, :, w0:w0 + W_CHUNK, :] = o.reshape(H, W_CHUNK, C)
    return out


_NC = None


def _get_nc():
    global _NC
    if _NC is None:
        _NC = build_nc()
    return _NC


def kernel(x, k):
    assert int(k) == 5
    x = np.asarray(x, dtype=np.float32)
    assert x.shape == (16, H, W, C)
    xb = np.ascontiguousarray(x.astype(ml_dtypes.bfloat16))
    nc = _get_nc()
    in_maps = [{"x": xb[2 * i:2 * i + 2]} for i in range(8)]
    res = run_bass_kernel_spmd(nc, in_maps, core_ids=list(range(8)))
    out = np.concatenate([np.asarray(r["out"]) for r in res.results], axis=0)
    return out.astype(np.float32)


# revision 27
# speedup vs baseline: 1.0431x; 1.0431x over previous
"""Trainium2 Bass kernel for 5x5 median filter (reflect padding, SAME size).

Input x: [16, 384, 384, 3] f32 (NHWC), k=5. Output: same shape.

Strategy:
- Pure data parallel over 8 NeuronCores: 2 images per core.
- All compute in bf16: DVE tensor_tensor runs in 2x_1P perf mode for
  16-bit dtypes (vs 1x for f32). Median selection commutes with the
  monotone f32->bf16 rounding, so the result equals round_bf16(true
  median): rel err <= 2^-9. Host converts f32<->bf16 at the edges.
- Per core layout: partition p = img*64 + hblock, each hblock = 6 output
  rows. Free dim = (10 input rows) x (100 px * 3 ch) for a 96-px chunk
  (2 px halo each side). 4 chunks cover W=384.
- 2x_1P needs 4B-aligned operands; a 1-px shift is 3 bf16 els = 6B.
  All DVE ops use even element offsets {0, 6}; odd shifts are
  materialized as shifted copies (s' = s<<1px, pm' = pm<<1px) on the
  otherwise-idle Scalar engine, ordered so they hide under DVE work.
- Median-of-25 via separable sorting network, emitted as FUSED
  multi-plane DVE instructions (same-ALU ops over several planes in
  one instruction via slot-major tiles and strided 4-dim APs) to
  amortize the ~150-cycle per-instruction overhead:
  1. vertical sort of 5-row columns (shared across horizontal windows)
  2. PM[x] = Batcher merge of sorted columns (x, x+1) -> sorted 10
  3. per window: L=PM[w-2], R=PM[w+1], M=sorted col w;
     u = mid-6 of merge(L,R) via DCE'd odd/even partial merges (the
     o/e partials are structurally identical -> fused 2-slot ops);
     median = rank 6 of merge(u, M).
- Reflect padding: row halos via DMAs from reflected rows, column halos
  via on-chip copies at image edges. DMAs round-robin over engine
  queues so chunk-0 issue latency shrinks.
"""

import numpy as np
import ml_dtypes

import concourse.bacc as bacc
import concourse.bass as bass
import concourse.mybir as mybir
from concourse.bass_utils import run_bass_kernel_spmd
from concourse.tile import TileContext

bf16 = mybir.dt.bfloat16
AMIN = mybir.AluOpType.min
AMAX = mybir.AluOpType.max

H = 384
W = 384
C = 3
ROW = W * C          # 1152 elements per image row
IMG = H * ROW        # elements per image
R = 6                # output rows per partition block
NBLK = H // R        # 64 blocks per image
W_CHUNK = 96         # output px per chunk
N_CHUNK = W // W_CHUNK

WS = (W_CHUNK + 4) * C    # column-sort domain width (els) = 300
WPM = 298                 # pair-merge op width (even, padded from 297)
WSEL = W_CHUNK * C        # selection/output domain width = 288
TW = 300                  # physical tile width for all work planes


# ---------------------------------------------------------------------------
# Emission context: bass backend + numpy simulation backend (for testing
# the op list without hardware).
# ---------------------------------------------------------------------------

class BassCtx:
    def __init__(self, nc, wp):
        self.nc = nc
        self.wp = wp
        self._tiles = {}

    def tile(self, tag, nslots):
        t = self.wp.tile([128, nslots * R, TW], bf16, tag=tag, name=tag)
        self._tiles[tag] = t
        return t

    # views ------------------------------------------------------------
    def sl(self, t, s0, ns=1, off=0, w=TW):
        """Contiguous slot range [s0, s0+ns), column window [off, off+w)."""
        return t[:, s0 * R:(s0 + ns) * R, off:off + w]

    def st(self, t, sl_, off=0, w=TW):
        """Strided slot view: sl_ is a python slice over slots."""
        r = t.rearrange("p (s r) w -> p s r w", r=R)
        return r[:, sl_, :, off:off + w]

    def xt_rows(self, xt, r0, nr, off=0, w=TW):
        return xt[:, r0:r0 + nr, off:off + w]

    def xt_l1(self, xt, r0, w=TW):
        """Rows {r0..r0+5} and {r0+3..r0+8} as a fused 2-slot view."""
        return bass.AP(xt.tensor, r0 * TW,
                       [[10 * TW, 128], [3 * TW, 2], [TW, R], [1, w]])

    # ops --------------------------------------------------------------
    def tt(self, op, out, in0, in1):
        self.nc.vector.tensor_tensor(out=out, in0=in0, in1=in1, op=op)

    def scopy(self, out, in_):
        self.nc.scalar.copy(out=out, in_=in_)

    def gcopy(self, out, in_):
        self.nc.gpsimd.tensor_copy(out, in_)

    def vcopy(self, out, in_):
        self.nc.vector.tensor_copy(out, in_)


class NumpyCtx:
    """Same op vocabulary over numpy arrays shaped [128, rows, TW]."""

    def __init__(self):
        self._tiles = {}

    def tile(self, tag, nslots):
        a = np.full((128, nslots * R, TW), np.nan, dtype=np.float32)
        self._tiles[tag] = a
        return a

    def sl(self, t, s0, ns=1, off=0, w=TW):
        return t[:, s0 * R:(s0 + ns) * R, off:off + w]

    def st(self, t, sl_, off=0, w=TW):
        r = t.reshape(128, -1, R, TW)
        return r[:, sl_, :, off:off + w]

    def xt_rows(self, xt, r0, nr, off=0, w=TW):
        return xt[:, r0:r0 + nr, off:off + w]

    def xt_l1(self, xt, r0, w=TW):
        v = np.lib.stride_tricks.as_strided(
            xt[:, r0:, :], shape=(128, 2, R, w),
            strides=(xt.strides[0], 3 * xt.strides[1], xt.strides[1],
                     xt.strides[2]))
        return v

    def tt(self, op, out, in0, in1):
        f = np.minimum if op is AMIN else np.maximum
        res = f(in0.reshape(out.shape), in1.reshape(out.shape))
        out[...] = res

    def scopy(self, out, in_):
        out[...] = in_.reshape(out.shape)

    gcopy = scopy
    vcopy = scopy


# ---------------------------------------------------------------------------
# One chunk: sort -> merge -> selection, with fused DVE ops and
# ScalarE shifted copies.
# ---------------------------------------------------------------------------

def emit_l1(ctx, xt, tiles):
    """First sort layer: CE(0,1), CE(3,4), fused. Emitted one chunk
    ahead so it fills the DVE stall while ScalarE finishes the s'
    copies of the previous chunk."""
    X2, X3 = tiles["X2"], tiles["X3"]
    ctx.tt(AMIN, ctx.sl(X2, 0, 2), ctx.xt_l1(xt, 0), ctx.xt_l1(xt, 1))
    ctx.tt(AMAX, ctx.sl(X3, 0, 2), ctx.xt_l1(xt, 0), ctx.xt_l1(xt, 1))


def emit_sort_rest(ctx, xt, tiles):
    S, SP, LH, Q, FP, X2, X3 = (tiles[k] for k in
                                ("S", "SP", "LH", "Q", "FP", "X2", "X3"))
    MIN, MAX = AMIN, AMAX
    sl = ctx.sl
    # X2 = (v0, v3), X3 = (v1, v4) from emit_l1.
    ctx.tt(MIN, sl(FP, 0), ctx.xt_rows(xt, 2, R), sl(X3, 1))       # (2,4)lo
    ctx.tt(MAX, sl(FP, 1), ctx.xt_rows(xt, 2, R), sl(X3, 1))       # (2,4)hi
    ctx.tt(MIN, sl(FP, 2), sl(FP, 0), sl(X2, 1))                   # (2,3)lo
    ctx.tt(MAX, sl(FP, 3), sl(FP, 0), sl(X2, 1))                   # (2,3)hi
    ctx.tt(MIN, sl(LH, 0), sl(X3, 0), sl(FP, 1))                   # (1,4)lo
    ctx.tt(MAX, sl(S, 4), sl(X3, 0), sl(FP, 1))                    # s4
    ctx.scopy(sl(SP, 4, 1, 0, WS - C), sl(S, 4, 1, C, WS - C))
    ctx.tt(MIN, sl(LH, 1), sl(X2, 0), sl(FP, 3))                   # (0,3)lo
    ctx.tt(MAX, sl(LH, 2), sl(X2, 0), sl(FP, 3))                   # (0,3)hi
    ctx.tt(MIN, sl(S, 0), sl(LH, 1), sl(FP, 2))                    # s0
    ctx.tt(MAX, sl(LH, 3), sl(LH, 1), sl(FP, 2))                   # (0,2)hi
    ctx.scopy(sl(SP, 0, 1, 0, WS - C), sl(S, 0, 1, C, WS - C))
    ctx.tt(MIN, sl(LH, 4), sl(LH, 0), sl(LH, 2))                   # (1,3)lo
    ctx.tt(MAX, sl(S, 3), sl(LH, 0), sl(LH, 2))                    # s3
    ctx.scopy(sl(SP, 3, 1, 0, WS - C), sl(S, 3, 1, C, WS - C))
    ctx.tt(MIN, sl(S, 1), sl(LH, 4), sl(LH, 3))                    # s1
    ctx.tt(MAX, sl(S, 2), sl(LH, 4), sl(LH, 3))                    # s2
    ctx.scopy(sl(SP, 1, 1, 0, WS - C), sl(S, 1, 1, C, WS - C))
    # s2's copy is on the critical path (last sort output): the DVE's
    # own 2x_2P tensor_copy (~1us) is cheaper than stalling on ScalarE
    ctx.vcopy(sl(SP, 2, 1, 0, WS - C), sl(S, 2, 1, C, WS - C))


def emit_merge_sel(ctx, outt_halves, tiles):
    S, SP, LH, PM, PMP, Q, FP = (tiles[k] for k in
                                 ("S", "SP", "LH", "PM", "PMP", "Q", "FP"))
    MIN, MAX = AMIN, AMAX
    sl, st = ctx.sl, ctx.st

    # ---- pair merge: PM[x] = merge(s[x], s[x+1]), b = s' ----
    # LH slots 0..4 = lo_i = min(a_i, b_i); slots 5..9 = hi_i.
    ctx.tt(MIN, sl(LH, 0, 5, 0, WPM), sl(S, 0, 5, 0, WPM),
           sl(SP, 0, 5, 0, WPM))
    ctx.tt(MAX, sl(LH, 5, 5, 0, WPM), sl(S, 0, 5, 0, WPM),
           sl(SP, 0, 5, 0, WPM))
    # pm0 = lo0, pm9 = hi4 (+ their shifted copies) on ScalarE
    ctx.scopy(sl(PM, 0, 1, 0, WPM), sl(LH, 0, 1, 0, WPM))
    ctx.scopy(sl(PMP, 0, 1, 0, WPM - C), sl(LH, 0, 1, C, WPM - C))
    ctx.scopy(sl(PM, 9, 1, 0, WPM), sl(LH, 9, 1, 0, WPM))
    ctx.scopy(sl(PMP, 9, 1, 0, WPM - C), sl(LH, 9, 1, C, WPM - C))
    # Q = (h1m, gm1, h2m, gm2): merges of (hi0,lo4) and (hi1,lo3)
    ctx.tt(MIN, sl(Q, 0, 2, 0, WPM), st(LH, slice(5, 7), 0, WPM),
           st(LH, slice(4, 2, -1), 0, WPM))
    ctx.tt(MAX, sl(Q, 2, 2, 0, WPM), st(LH, slice(5, 7), 0, WPM),
           st(LH, slice(4, 2, -1), 0, WPM))
    # FP = (f1, f2, f3, f4): f1,f2 = merge(lo2, h1m); f3,f4 = (hi2, h2m)
    ctx.tt(MIN, st(FP, slice(0, 3, 2), 0, WPM),
           st(LH, slice(2, 8, 5), 0, WPM), st(Q, slice(0, 3, 2), 0, WPM))
    ctx.tt(MAX, st(FP, slice(1, 4, 2), 0, WPM),
           st(LH, slice(2, 8, 5), 0, WPM), st(Q, slice(0, 3, 2), 0, WPM))
    # loop: pm_{2i+1} = min(g_i, f_{i+1}); pm_{2i+2} = max.
    # g = (lo1, gm1, gm2, hi3) = (LH1, Q1, Q3, LH8); f = FP0..FP3.
    # singles, ordered so the ScalarE pm' copies (same order) finish
    # before the selection needs them: {2,3} first, then {6,7},{4,5},
    # {0,1},{8,9} pairs.
    g_ops = [  # (pm_idx, op, g_view, f_view)
        (2, MAX, sl(LH, 1, 1, 0, WPM), sl(FP, 0, 1, 0, WPM)),
        (3, MIN, sl(Q, 1, 1, 0, WPM), sl(FP, 1, 1, 0, WPM)),
        (6, MAX, sl(Q, 3, 1, 0, WPM), sl(FP, 2, 1, 0, WPM)),
        (7, MIN, sl(LH, 8, 1, 0, WPM), sl(FP, 3, 1, 0, WPM)),
        (4, MAX, sl(Q, 1, 1, 0, WPM), sl(FP, 1, 1, 0, WPM)),
        (5, MIN, sl(Q, 3, 1, 0, WPM), sl(FP, 2, 1, 0, WPM)),
        (1, MIN, sl(LH, 1, 1, 0, WPM), sl(FP, 0, 1, 0, WPM)),
        (8, MAX, sl(LH, 8, 1, 0, WPM), sl(FP, 3, 1, 0, WPM)),
    ]
    for pi, op, gv, fv in g_ops:
        ctx.tt(op, sl(PM, pi, 1, 0, WPM), gv, fv)
        ctx.scopy(sl(PMP, pi, 1, 0, WPM - C), sl(PM, pi, 1, C, WPM - C))

    # ---- selection: mid-6 of merge(L,R), then rank 6 of merge(u, M) --
    # Fused 2-slot ops; sub-slot 0 = e-variant (even pm), 1 = o-variant.
    # Operand pairs: t1:{2,3} t2:{6,7} k:{4,5} t3:{0,1} t4:{8,9}.
    def pmpair(i):
        return (sl(PM, 2 * i, 2, 0, WSEL), sl(PMP, 2 * i, 2, 2 * C, WSEL))

    W2 = WSEL
    t1, t1b = sl(SP, 0, 2, 0, W2), pmpair(1)
    ctx.tt(MAX, t1, *t1b)
    t2 = sl(SP, 2, 2, 0, W2)
    ctx.tt(MIN, t2, *pmpair(3))
    g1 = sl(LH, 0, 2, 0, W2)
    ctx.tt(MIN, g1, t1, t2)
    g2 = sl(LH, 2, 2, 0, W2)
    ctx.tt(MAX, g2, t1, t2)
    k0 = sl(LH, 5, 2, 0, W2)
    ctx.tt(MIN, k0, *pmpair(2))
    k1 = sl(LH, 7, 2, 0, W2)
    ctx.tt(MAX, k1, *pmpair(2))
    t3 = sl(Q, 0, 2, 0, W2)
    ctx.tt(MAX, t3, *pmpair(0))
    t4 = sl(Q, 2, 2, 0, W2)
    ctx.tt(MIN, t4, *pmpair(4))
    h1 = sl(FP, 0, 2, 0, W2)
    ctx.tt(MIN, h1, t3, t4)
    h2 = sl(FP, 2, 2, 0, W2)
    ctx.tt(MAX, h2, t3, t4)
    f2 = sl(SP, 0, 2, 0, W2)      # overwrites t1 (dead)
    ctx.tt(MAX, f2, k0, h1)
    f3 = sl(SP, 2, 2, 0, W2)      # overwrites t2 (dead)
    ctx.tt(MIN, f3, k1, h2)

    # singles: o3,o4,o5 -> LH slots 5,6,7 ; e4,e5,e6 -> Q slots 0,1,2
    def sub(v2, j):  # sub-slot j of a 2-slot view
        return v2[:, j * R:(j + 1) * R, :]

    ctx.tt(MIN, sl(LH, 5, 1, 0, W2), sub(g1, 1), sub(f2, 1))   # o3
    ctx.tt(MAX, sl(LH, 6, 1, 0, W2), sub(g1, 1), sub(f2, 1))   # o4
    ctx.tt(MIN, sl(LH, 7, 1, 0, W2), sub(g2, 1), sub(f3, 1))   # o5
    ctx.tt(MAX, sl(Q, 0, 1, 0, W2), sub(g1, 0), sub(f2, 0))    # e4
    ctx.tt(MIN, sl(Q, 1, 1, 0, W2), sub(g2, 0), sub(f3, 0))    # e5
    ctx.tt(MAX, sl(Q, 2, 1, 0, W2), sub(g2, 0), sub(f3, 0))    # e6

    # u_i -> PM slots 0..5 (dead by now): u0,u2,u4 = min(o,e) strided
    ctx.tt(MIN, st(PM, slice(0, 6, 2), 0, W2), sl(LH, 5, 3, 0, W2),
           sl(Q, 0, 3, 0, W2))
    ctx.tt(MAX, st(PM, slice(1, 6, 2), 0, W2), sl(LH, 5, 3, 0, W2),
           sl(Q, 0, 3, 0, W2))

    # final: fused (x2,x1,k1p) = max(u_i, M_i) i=0..2 ;
    #        (q0,x3) = min(u_i, M_i) i=3..4
    ctx.tt(MAX, sl(FP, 0, 3, 0, W2), sl(PM, 0, 3, 0, W2),
           sl(S, 0, 3, 2 * C, W2))
    ctx.tt(MIN, sl(Q, 0, 2, 0, W2), sl(PM, 3, 2, 0, W2),
           sl(S, 3, 2, 2 * C, W2))
    ctx.tt(MIN, sl(LH, 0, 1, 0, W2), sl(FP, 1, 1, 0, W2),
           sl(PM, 5, 1, 0, W2))                                 # p1
    ctx.tt(MAX, sl(LH, 1, 1, 0, W2), sl(Q, 0, 1, 0, W2),
           sl(LH, 0, 1, 0, W2))                                 # o2p
    ctx.tt(MAX, sl(LH, 2, 1, 0, W2), sl(FP, 0, 1, 0, W2),
           sl(Q, 1, 1, 0, W2))                                  # h2p
    ctx.tt(MIN, sl(LH, 3, 1, 0, W2), sl(FP, 2, 1, 0, W2),
           sl(LH, 2, 1, 0, W2))                                 # e3p
    outv, post = outt_halves
    ctx.tt(MIN, outv, sl(LH, 1, 1, 0, W2), sl(LH, 3, 1, 0, W2))  # median
    post()


# ---------------------------------------------------------------------------
# Kernel builder
# ---------------------------------------------------------------------------

def build_nc():
    nc = bacc.Bacc("TRN2", target_bir_lowering=False)
    x = nc.dram_tensor("x", [2, H, W, C], bf16, kind="ExternalInput")
    y = nc.dram_tensor("out", [2, H, W, C], bf16, kind="ExternalOutput")
    dma_engs0 = [nc.sync, nc.gpsimd, nc.scalar]
    dma_engs = [nc.sync, nc.gpsimd]

    with TileContext(nc) as tc:
        with tc.tile_pool(name="io", bufs=2) as iop, \
             tc.tile_pool(name="work", bufs=1) as wp:
            ctx = BassCtx(nc, wp)
            tiles = {k: ctx.tile(k, n) for k, n in
                     (("S", 5), ("SP", 5), ("LH", 10), ("PM", 10),
                      ("PMP", 10), ("Q", 4), ("FP", 4), ("X2", 2),
                      ("X3", 2))}
            qi = 0

            def dma(out, in_, engs):
                nonlocal qi
                engs[qi % len(engs)].dma_start(out=out, in_=in_)
                qi += 1

            def emit_load(ci):
                w0 = ci * W_CHUNK
                pxlo = max(0, w0 - 2)
                pxhi = min(W, w0 + W_CHUNK + 2)
                n = (pxhi - pxlo) * C
                elo = (pxlo - (w0 - 2)) * C
                engs = dma_engs0 if ci == 0 else dma_engs
                xt = iop.tile([128, 10, WS], bf16, tag="xt", name="xt")
                for img in range(2):
                    base = img * IMG + pxlo * C
                    p0 = img * NBLK
                    # chunk 0 is latency-critical (nothing to overlap):
                    # split finer so transfers parallelize across rings
                    splits = [1, 33, 63]
                    for si in range(len(splits) - 1):
                        h0, h1 = splits[si], splits[si + 1]
                        src = bass.AP(x, base + (6 * h0 - 2) * ROW,
                                      [[6 * ROW, h1 - h0], [ROW, 10], [1, n]])
                        dma(xt[p0 + h0:p0 + h1, :, elo:elo + n], src, engs)
                    src = bass.AP(x, base, [[ROW, 1], [ROW, 8], [1, n]])
                    dma(xt[p0:p0 + 1, 2:10, elo:elo + n], src, engs)
                    for j, r in ((0, 2), (1, 1)):
                        src = bass.AP(x, base + r * ROW, [[ROW, 1], [1, n]])
                        dma(xt[p0:p0 + 1, j:j + 1, elo:elo + n], src, engs)
                    p63 = p0 + NBLK - 1
                    src = bass.AP(x, base + 376 * ROW,
                                  [[ROW, 1], [ROW, 8], [1, n]])
                    dma(xt[p63:p63 + 1, 0:8, elo:elo + n], src, engs)
                    for j, r in ((8, 382), (9, 381)):
                        src = bass.AP(x, base + r * ROW, [[ROW, 1], [1, n]])
                        dma(xt[p63:p63 + 1, j:j + 1, elo:elo + n], src, engs)

                if ci == 0:
                    nc.scalar.copy(out=xt[:, :, 0:C],
                                   in_=xt[:, :, 4 * C:5 * C])
                    nc.scalar.copy(out=xt[:, :, C:2 * C],
                                   in_=xt[:, :, 3 * C:4 * C])
                if ci == N_CHUNK - 1:
                    wc = W_CHUNK
                    nc.scalar.copy(out=xt[:, :, (wc + 2) * C:(wc + 3) * C],
                                   in_=xt[:, :, wc * C:(wc + 1) * C])
                    nc.scalar.copy(out=xt[:, :, (wc + 3) * C:(wc + 4) * C],
                                   in_=xt[:, :, (wc - 1) * C:wc * C])
                return xt

            xt = emit_load(0)
            emit_l1(ctx, xt, tiles)
            for ci in range(N_CHUNK):
                w0 = ci * W_CHUNK
                emit_sort_rest(ctx, xt, tiles)
                if ci + 1 < N_CHUNK:
                    xt = emit_load(ci + 1)
                    emit_l1(ctx, xt, tiles)

                outt = iop.tile([128, R, WSEL], bf16, tag="outt", name="outt",
                                bufs=1)

                def post(outt=outt, w0=w0):
                    oengs = [nc.sync, nc.gpsimd]
                    for img in range(2):
                        p0 = img * NBLK
                        half = NBLK // 2
                        for hs in range(2):
                            dst = bass.AP(
                                y, img * IMG + hs * half * R * ROW + w0 * C,
                                [[R * ROW, half], [ROW, R], [1, WSEL]])
                            dma(dst,
                                outt[p0 + hs * half:p0 + (hs + 1) * half,
                                     :, :], oengs)

                emit_merge_sel(ctx, (outt[:], post), tiles)

    nc.finalize()
    return nc


# ---------------------------------------------------------------------------
# Numpy simulation of one core (for offline verification of the op list)
# ---------------------------------------------------------------------------

def simulate_core(x2):
    """x2: [2, H, W, C] float32 (pre-rounded to bf16 grid). Returns
    [2, H, W, C] median-filter output computed via the exact op list."""
    out = np.zeros_like(x2)
    xp = np.pad(x2, ((0, 0), (2, 2), (0, 0), (0, 0)), mode="reflect")
    for ci in range(N_CHUNK):
        w0 = ci * W_CHUNK
        pxlo = max(0, w0 - 2)
        pxhi = min(W, w0 + W_CHUNK + 2)
        n = (pxhi - pxlo) * C
        elo = (pxlo - (w0 - 2)) * C

        ctx = NumpyCtx()
        tiles = {k: ctx.tile(k, nsl) for k, nsl in
                 (("S", 5), ("SP", 5), ("LH", 10), ("PM", 10),
                  ("PMP", 10), ("Q", 4), ("FP", 4), ("X2", 2), ("X3", 2))}
        xt = np.full((128, 10, WS), np.nan, dtype=np.float32)
        for img in range(2):
            p0 = img * NBLK
            rows = xp[img].reshape(H + 4, ROW)
            for hb in range(NBLK):
                r0 = hb * R  # padded-row index of first input row
                xt[p0 + hb, :, elo:elo + n] = \
                    rows[r0:r0 + 10, pxlo * C:pxlo * C + n]
        if ci == 0:
            xt[:, :, 0:C] = xt[:, :, 4 * C:5 * C]
            xt[:, :, C:2 * C] = xt[:, :, 3 * C:4 * C]
        if ci == N_CHUNK - 1:
            wc = W_CHUNK
            xt[:, :, (wc + 2) * C:(wc + 3) * C] = xt[:, :, wc * C:(wc + 1) * C]
            xt[:, :, (wc + 3) * C:(wc + 4) * C] = \
                xt[:, :, (wc - 1) * C:wc * C]

        outt = np.full((128, R, WSEL), np.nan, dtype=np.float32)
        emit_l1(ctx, xt, tiles)
        emit_sort_rest(ctx, xt, tiles)
        emit_merge_sel(ctx, (outt, lambda: None), tiles)

        for img in range(2):
            p0 = img * NBLK
            o = outt[p0:p0 + NBLK].reshape(H, WSEL)
            out[img, :, w0:w0 + W_CHUNK, :] = o.reshape(H, W_CHUNK, C)
    return out


_NC = None


def _get_nc():
    global _NC
    if _NC is None:
        _NC = build_nc()
    return _NC


def kernel(x, k):
    assert int(k) == 5
    x = np.asarray(x, dtype=np.float32)
    assert x.shape == (16, H, W, C)
    xb = np.ascontiguousarray(x.astype(ml_dtypes.bfloat16))
    nc = _get_nc()
    in_maps = [{"x": xb[2 * i:2 * i + 2]} for i in range(8)]
    res = run_bass_kernel_spmd(nc, in_maps, core_ids=list(range(8)))
    out = np.concatenate([np.asarray(r["out"]) for r in res.results], axis=0)
    return out.astype(np.float32)


# revision 29
# speedup vs baseline: 1.0447x; 1.0015x over previous
"""Trainium2 Bass kernel for 5x5 median filter (reflect padding, SAME size).

Input x: [16, 384, 384, 3] f32 (NHWC), k=5. Output: same shape.

Strategy:
- Pure data parallel over 8 NeuronCores: 2 images per core.
- All compute in bf16: DVE tensor_tensor runs in 2x_1P perf mode for
  16-bit dtypes (vs 1x for f32). Median selection commutes with the
  monotone f32->bf16 rounding, so the result equals round_bf16(true
  median): rel err <= 2^-9. Host converts f32<->bf16 at the edges.
- Per core layout: partition p = img*64 + hblock, each hblock = 6 output
  rows. Free dim = (10 input rows) x (100 px * 3 ch) for a 96-px chunk
  (2 px halo each side). 4 chunks cover W=384.
- 2x_1P needs 4B-aligned operands; a 1-px shift is 3 bf16 els = 6B.
  All DVE ops use even element offsets {0, 6}; odd shifts are
  materialized as shifted copies (s' = s<<1px, pm' = pm<<1px) on the
  otherwise-idle Scalar engine, ordered so they hide under DVE work.
- Median-of-25 via separable sorting network, emitted as FUSED
  multi-plane DVE instructions (same-ALU ops over several planes in
  one instruction via slot-major tiles and strided 4-dim APs) to
  amortize the ~150-cycle per-instruction overhead:
  1. vertical sort of 5-row columns (shared across horizontal windows)
  2. PM[x] = Batcher merge of sorted columns (x, x+1) -> sorted 10
  3. per window: L=PM[w-2], R=PM[w+1], M=sorted col w;
     u = mid-6 of merge(L,R) via DCE'd odd/even partial merges (the
     o/e partials are structurally identical -> fused 2-slot ops);
     median = rank 6 of merge(u, M).
- Reflect padding: row halos via DMAs from reflected rows, column halos
  via on-chip copies at image edges. DMAs round-robin over engine
  queues so chunk-0 issue latency shrinks.
"""

import numpy as np
import ml_dtypes

import concourse.bacc as bacc
import concourse.bass as bass
import concourse.mybir as mybir
from concourse.bass_utils import run_bass_kernel_spmd
from concourse.tile import TileContext

bf16 = mybir.dt.bfloat16
AMIN = mybir.AluOpType.min
AMAX = mybir.AluOpType.max

H = 384
W = 384
C = 3
ROW = W * C          # 1152 elements per image row
IMG = H * ROW        # elements per image
R = 6                # output rows per partition block
NBLK = H // R        # 64 blocks per image
W_CHUNK = 96         # output px per chunk
N_CHUNK = W // W_CHUNK

WS = (W_CHUNK + 4) * C    # column-sort domain width (els) = 300
WPM = 298                 # pair-merge op width (even, padded from 297)
WSEL = W_CHUNK * C        # selection/output domain width = 288
TW = 300                  # physical tile width for all work planes


# ---------------------------------------------------------------------------
# Emission context: bass backend + numpy simulation backend (for testing
# the op list without hardware).
# ---------------------------------------------------------------------------

class BassCtx:
    def __init__(self, nc, wp):
        self.nc = nc
        self.wp = wp
        self._tiles = {}

    def tile(self, tag, nslots):
        t = self.wp.tile([128, nslots * R, TW], bf16, tag=tag, name=tag)
        self._tiles[tag] = t
        return t

    # views ------------------------------------------------------------
    def sl(self, t, s0, ns=1, off=0, w=TW):
        """Contiguous slot range [s0, s0+ns), column window [off, off+w)."""
        return t[:, s0 * R:(s0 + ns) * R, off:off + w]

    def st(self, t, sl_, off=0, w=TW):
        """Strided slot view: sl_ is a python slice over slots."""
        r = t.rearrange("p (s r) w -> p s r w", r=R)
        return r[:, sl_, :, off:off + w]

    def xt_rows(self, xt, r0, nr, off=0, w=TW):
        return xt[:, r0:r0 + nr, off:off + w]

    def xt_l1(self, xt, r0, w=TW):
        """Rows {r0..r0+5} and {r0+3..r0+8} as a fused 2-slot view."""
        return bass.AP(xt.tensor, r0 * TW,
                       [[10 * TW, 128], [3 * TW, 2], [TW, R], [1, w]])

    # ops --------------------------------------------------------------
    def tt(self, op, out, in0, in1):
        self.nc.vector.tensor_tensor(out=out, in0=in0, in1=in1, op=op)

    def scopy(self, out, in_):
        self.nc.scalar.copy(out=out, in_=in_)

    def gcopy(self, out, in_):
        self.nc.gpsimd.tensor_copy(out, in_)

    def vcopy(self, out, in_):
        self.nc.vector.tensor_copy(out, in_)


class NumpyCtx:
    """Same op vocabulary over numpy arrays shaped [128, rows, TW]."""

    def __init__(self):
        self._tiles = {}

    def tile(self, tag, nslots):
        a = np.full((128, nslots * R, TW), np.nan, dtype=np.float32)
        self._tiles[tag] = a
        return a

    def sl(self, t, s0, ns=1, off=0, w=TW):
        return t[:, s0 * R:(s0 + ns) * R, off:off + w]

    def st(self, t, sl_, off=0, w=TW):
        r = t.reshape(128, -1, R, TW)
        return r[:, sl_, :, off:off + w]

    def xt_rows(self, xt, r0, nr, off=0, w=TW):
        return xt[:, r0:r0 + nr, off:off + w]

    def xt_l1(self, xt, r0, w=TW):
        v = np.lib.stride_tricks.as_strided(
            xt[:, r0:, :], shape=(128, 2, R, w),
            strides=(xt.strides[0], 3 * xt.strides[1], xt.strides[1],
                     xt.strides[2]))
        return v

    def tt(self, op, out, in0, in1):
        f = np.minimum if op is AMIN else np.maximum
        res = f(in0.reshape(out.shape), in1.reshape(out.shape))
        out[...] = res

    def scopy(self, out, in_):
        out[...] = in_.reshape(out.shape)

    gcopy = scopy
    vcopy = scopy


# ---------------------------------------------------------------------------
# One chunk: sort -> merge -> selection, with fused DVE ops and
# ScalarE shifted copies.
# ---------------------------------------------------------------------------

def emit_p2(ctx, xt, tiles):
    """Shared sorted-pairs of vertically adjacent rows: P2[i] =
    CE(row i, row i+1) for i=0..8, as two fused 9-row ops. Each pair
    is reused by the sorted-triple above it and the tail pair below
    (sliding-window sharing). Emitted one chunk ahead so it fills the
    DVE stall while ScalarE finishes the previous chunk's s' copies."""
    X2, X3 = tiles["X2"], tiles["X3"]
    ctx.tt(AMIN, X2[:, 0:9, :], ctx.xt_rows(xt, 0, 9),
           ctx.xt_rows(xt, 1, 9))
    ctx.tt(AMAX, X3[:, 0:9, :], ctx.xt_rows(xt, 0, 9),
           ctx.xt_rows(xt, 1, 9))


def emit_sort_rest(ctx, xt, tiles):
    """sorted5[r] = merge(sorted3(rows r..r+2), pair(rows r+3, r+4))
    via odd-even merge(3,2). sorted3 = insert row r+2 into P2[r]."""
    S, SP, Q, FP, X2, X3 = (tiles[k] for k in
                            ("S", "SP", "Q", "FP", "X2", "X3"))
    MIN, MAX = AMIN, AMAX
    sl, st = ctx.sl, ctx.st
    x6 = ctx.xt_rows(xt, 2, R)
    p2lo, p2hi = X2[:, 0:R, :], X3[:, 0:R, :]
    b0, b1 = X2[:, 3:3 + R, :], X3[:, 3:3 + R, :]
    ctx.tt(MIN, sl(Q, 0), x6, p2lo)                   # a0
    ctx.tt(MAX, sl(Q, 1), x6, p2lo)                   # t
    ctx.tt(MIN, sl(Q, 2), sl(Q, 1), p2hi)             # a1
    ctx.tt(MAX, sl(Q, 3), sl(Q, 1), p2hi)             # a2
    ctx.tt(MIN, sl(S, 0), b0, sl(Q, 0))               # s0
    ctx.scopy(sl(SP, 0, 1, 0, WS - C), sl(S, 0, 1, C, WS - C))
    ctx.tt(MAX, sl(Q, 1), b0, sl(Q, 0))               # t' (odd-merge)
    ctx.tt(MIN, sl(FP, 0), sl(Q, 1), sl(Q, 3))        # z1
    ctx.tt(MAX, sl(FP, 1), sl(Q, 1), sl(Q, 3))        # z2
    ctx.tt(MIN, sl(FP, 2), sl(Q, 2), b1)              # d0
    ctx.tt(MAX, sl(FP, 3), sl(Q, 2), b1)              # d1
    ctx.tt(MIN, st(S, slice(1, 4, 2)), sl(FP, 2, 2), sl(FP, 0, 2))  # s1,s3
    ctx.tt(MAX, st(S, slice(2, 5, 2)), sl(FP, 2, 2), sl(FP, 0, 2))  # s2,s4
    # 4 of 5 sorted outputs land in the last two (fused) ops; put 3 of
    # the shifted copies on the DVE itself (fast 2x/4x tensor_copy) so
    # ScalarE's serial chain doesn't stall the merge
    ctx.scopy(sl(SP, 4, 1, 0, WS - C), sl(S, 4, 1, C, WS - C))
    ctx.vcopy(sl(SP, 1, 1, 0, WS - C), sl(S, 1, 1, C, WS - C))
    ctx.vcopy(sl(SP, 2, 1, 0, WS - C), sl(S, 2, 1, C, WS - C))
    ctx.vcopy(sl(SP, 3, 1, 0, WS - C), sl(S, 3, 1, C, WS - C))


def emit_merge_sel(ctx, outt_halves, tiles):
    S, SP, LH, PM, PMP, Q, FP = (tiles[k] for k in
                                 ("S", "SP", "LH", "PM", "PMP", "Q", "FP"))
    MIN, MAX = AMIN, AMAX
    sl, st = ctx.sl, ctx.st

    # ---- pair merge: PM[x] = merge(s[x], s[x+1]), b = s' ----
    # LH slots 0..4 = lo_i = min(a_i, b_i); slots 5..9 = hi_i.
    ctx.tt(MIN, sl(LH, 0, 5, 0, WPM), sl(S, 0, 5, 0, WPM),
           sl(SP, 0, 5, 0, WPM))
    ctx.tt(MAX, sl(LH, 5, 5, 0, WPM), sl(S, 0, 5, 0, WPM),
           sl(SP, 0, 5, 0, WPM))
    # pm0 = lo0, pm9 = hi4 (+ their shifted copies) on ScalarE
    ctx.scopy(sl(PM, 0, 1, 0, WPM), sl(LH, 0, 1, 0, WPM))
    ctx.scopy(sl(PMP, 0, 1, 0, WPM - C), sl(LH, 0, 1, C, WPM - C))
    ctx.scopy(sl(PM, 9, 1, 0, WPM), sl(LH, 9, 1, 0, WPM))
    ctx.scopy(sl(PMP, 9, 1, 0, WPM - C), sl(LH, 9, 1, C, WPM - C))
    # Q = (h1m, gm1, h2m, gm2): merges of (hi0,lo4) and (hi1,lo3)
    ctx.tt(MIN, sl(Q, 0, 2, 0, WPM), st(LH, slice(5, 7), 0, WPM),
           st(LH, slice(4, 2, -1), 0, WPM))
    ctx.tt(MAX, sl(Q, 2, 2, 0, WPM), st(LH, slice(5, 7), 0, WPM),
           st(LH, slice(4, 2, -1), 0, WPM))
    # FP = (f1, f2, f3, f4): f1,f2 = merge(lo2, h1m); f3,f4 = (hi2, h2m)
    ctx.tt(MIN, st(FP, slice(0, 3, 2), 0, WPM),
           st(LH, slice(2, 8, 5), 0, WPM), st(Q, slice(0, 3, 2), 0, WPM))
    ctx.tt(MAX, st(FP, slice(1, 4, 2), 0, WPM),
           st(LH, slice(2, 8, 5), 0, WPM), st(Q, slice(0, 3, 2), 0, WPM))
    # loop: pm_{2i+1} = min(g_i, f_{i+1}); pm_{2i+2} = max.
    # g = (lo1, gm1, gm2, hi3) = (LH1, Q1, Q3, LH8); f = FP0..FP3.
    # singles, ordered so the ScalarE pm' copies (same order) finish
    # before the selection needs them: {2,3} first, then {6,7},{4,5},
    # {0,1},{8,9} pairs.
    g_ops = [  # (pm_idx, op, g_view, f_view)
        (2, MAX, sl(LH, 1, 1, 0, WPM), sl(FP, 0, 1, 0, WPM)),
        (3, MIN, sl(Q, 1, 1, 0, WPM), sl(FP, 1, 1, 0, WPM)),
        (6, MAX, sl(Q, 3, 1, 0, WPM), sl(FP, 2, 1, 0, WPM)),
        (7, MIN, sl(LH, 8, 1, 0, WPM), sl(FP, 3, 1, 0, WPM)),
        (4, MAX, sl(Q, 1, 1, 0, WPM), sl(FP, 1, 1, 0, WPM)),
        (5, MIN, sl(Q, 3, 1, 0, WPM), sl(FP, 2, 1, 0, WPM)),
        (1, MIN, sl(LH, 1, 1, 0, WPM), sl(FP, 0, 1, 0, WPM)),
        (8, MAX, sl(LH, 8, 1, 0, WPM), sl(FP, 3, 1, 0, WPM)),
    ]
    for pi, op, gv, fv in g_ops:
        ctx.tt(op, sl(PM, pi, 1, 0, WPM), gv, fv)
        ctx.scopy(sl(PMP, pi, 1, 0, WPM - C), sl(PM, pi, 1, C, WPM - C))

    # ---- selection: mid-6 of merge(L,R), then rank 6 of merge(u, M) --
    # Fused 2-slot ops; sub-slot 0 = e-variant (even pm), 1 = o-variant.
    # Operand pairs: t1:{2,3} t2:{6,7} k:{4,5} t3:{0,1} t4:{8,9}.
    def pmpair(i):
        return (sl(PM, 2 * i, 2, 0, WSEL), sl(PMP, 2 * i, 2, 2 * C, WSEL))

    W2 = WSEL
    t1, t1b = sl(SP, 0, 2, 0, W2), pmpair(1)
    ctx.tt(MAX, t1, *t1b)
    t2 = sl(SP, 2, 2, 0, W2)
    ctx.tt(MIN, t2, *pmpair(3))
    g1 = sl(LH, 0, 2, 0, W2)
    ctx.tt(MIN, g1, t1, t2)
    g2 = sl(LH, 2, 2, 0, W2)
    ctx.tt(MAX, g2, t1, t2)
    k0 = sl(LH, 5, 2, 0, W2)
    ctx.tt(MIN, k0, *pmpair(2))
    k1 = sl(LH, 7, 2, 0, W2)
    ctx.tt(MAX, k1, *pmpair(2))
    t3 = sl(Q, 0, 2, 0, W2)
    ctx.tt(MAX, t3, *pmpair(0))
    t4 = sl(Q, 2, 2, 0, W2)
    ctx.tt(MIN, t4, *pmpair(4))
    h1 = sl(FP, 0, 2, 0, W2)
    ctx.tt(MIN, h1, t3, t4)
    h2 = sl(FP, 2, 2, 0, W2)
    ctx.tt(MAX, h2, t3, t4)
    f2 = sl(SP, 0, 2, 0, W2)      # overwrites t1 (dead)
    ctx.tt(MAX, f2, k0, h1)
    f3 = sl(SP, 2, 2, 0, W2)      # overwrites t2 (dead)
    ctx.tt(MIN, f3, k1, h2)

    # singles: o3,o4,o5 -> LH slots 5,6,7 ; e4,e5,e6 -> Q slots 0,1,2
    def sub(v2, j):  # sub-slot j of a 2-slot view
        return v2[:, j * R:(j + 1) * R, :]

    ctx.tt(MIN, sl(LH, 5, 1, 0, W2), sub(g1, 1), sub(f2, 1))   # o3
    ctx.tt(MAX, sl(LH, 6, 1, 0, W2), sub(g1, 1), sub(f2, 1))   # o4
    ctx.tt(MIN, sl(LH, 7, 1, 0, W2), sub(g2, 1), sub(f3, 1))   # o5
    ctx.tt(MAX, sl(Q, 0, 1, 0, W2), sub(g1, 0), sub(f2, 0))    # e4
    ctx.tt(MIN, sl(Q, 1, 1, 0, W2), sub(g2, 0), sub(f3, 0))    # e5
    ctx.tt(MAX, sl(Q, 2, 1, 0, W2), sub(g2, 0), sub(f3, 0))    # e6

    # u_i -> PM slots 0..5 (dead by now): u0,u2,u4 = min(o,e) strided
    ctx.tt(MIN, st(PM, slice(0, 6, 2), 0, W2), sl(LH, 5, 3, 0, W2),
           sl(Q, 0, 3, 0, W2))
    ctx.tt(MAX, st(PM, slice(1, 6, 2), 0, W2), sl(LH, 5, 3, 0, W2),
           sl(Q, 0, 3, 0, W2))

    # final: fused (x2,x1,k1p) = max(u_i, M_i) i=0..2 ;
    #        (q0,x3) = min(u_i, M_i) i=3..4
    ctx.tt(MAX, sl(FP, 0, 3, 0, W2), sl(PM, 0, 3, 0, W2),
           sl(S, 0, 3, 2 * C, W2))
    ctx.tt(MIN, sl(Q, 0, 2, 0, W2), sl(PM, 3, 2, 0, W2),
           sl(S, 3, 2, 2 * C, W2))
    ctx.tt(MIN, sl(LH, 0, 1, 0, W2), sl(FP, 1, 1, 0, W2),
           sl(PM, 5, 1, 0, W2))                                 # p1
    ctx.tt(MAX, sl(LH, 1, 1, 0, W2), sl(Q, 0, 1, 0, W2),
           sl(LH, 0, 1, 0, W2))                                 # o2p
    ctx.tt(MAX, sl(LH, 2, 1, 0, W2), sl(FP, 0, 1, 0, W2),
           sl(Q, 1, 1, 0, W2))                                  # h2p
    ctx.tt(MIN, sl(LH, 3, 1, 0, W2), sl(FP, 2, 1, 0, W2),
           sl(LH, 2, 1, 0, W2))                                 # e3p
    outv, post = outt_halves
    ctx.tt(MIN, outv, sl(LH, 1, 1, 0, W2), sl(LH, 3, 1, 0, W2))  # median
    post()


# ---------------------------------------------------------------------------
# Kernel builder
# ---------------------------------------------------------------------------

def build_nc():
    nc = bacc.Bacc("TRN2", target_bir_lowering=False)
    x = nc.dram_tensor("x", [2, H, W, C], bf16, kind="ExternalInput")
    y = nc.dram_tensor("out", [2, H, W, C], bf16, kind="ExternalOutput")
    dma_engs0 = [nc.sync, nc.gpsimd, nc.scalar]
    dma_engs = [nc.sync, nc.gpsimd]

    with TileContext(nc) as tc:
        with tc.tile_pool(name="io", bufs=2) as iop, \
             tc.tile_pool(name="work", bufs=1) as wp:
            ctx = BassCtx(nc, wp)
            tiles = {k: ctx.tile(k, n) for k, n in
                     (("S", 5), ("SP", 5), ("LH", 10), ("PM", 10),
                      ("PMP", 10), ("Q", 4), ("FP", 4), ("X2", 2),
                      ("X3", 2))}
            qi = 0

            def dma(out, in_, engs):
                nonlocal qi
                engs[qi % len(engs)].dma_start(out=out, in_=in_)
                qi += 1

            def emit_load(ci):
                w0 = ci * W_CHUNK
                pxlo = max(0, w0 - 2)
                pxhi = min(W, w0 + W_CHUNK + 2)
                n = (pxhi - pxlo) * C
                elo = (pxlo - (w0 - 2)) * C
                engs = dma_engs0 if ci == 0 else dma_engs
                xt = iop.tile([128, 10, WS], bf16, tag="xt", name="xt")
                for img in range(2):
                    base = img * IMG + pxlo * C
                    p0 = img * NBLK
                    # chunk 0 is latency-critical (nothing to overlap):
                    # split finer so transfers parallelize across rings
                    splits = [1, 33, 63]
                    for si in range(len(splits) - 1):
                        h0, h1 = splits[si], splits[si + 1]
                        src = bass.AP(x, base + (6 * h0 - 2) * ROW,
                                      [[6 * ROW, h1 - h0], [ROW, 10], [1, n]])
                        dma(xt[p0 + h0:p0 + h1, :, elo:elo + n], src, engs)
                    src = bass.AP(x, base, [[ROW, 1], [ROW, 8], [1, n]])
                    dma(xt[p0:p0 + 1, 2:10, elo:elo + n], src, engs)
                    for j, r in ((0, 2), (1, 1)):
                        src = bass.AP(x, base + r * ROW, [[ROW, 1], [1, n]])
                        dma(xt[p0:p0 + 1, j:j + 1, elo:elo + n], src, engs)
                    p63 = p0 + NBLK - 1
                    src = bass.AP(x, base + 376 * ROW,
                                  [[ROW, 1], [ROW, 8], [1, n]])
                    dma(xt[p63:p63 + 1, 0:8, elo:elo + n], src, engs)
                    for j, r in ((8, 382), (9, 381)):
                        src = bass.AP(x, base + r * ROW, [[ROW, 1], [1, n]])
                        dma(xt[p63:p63 + 1, j:j + 1, elo:elo + n], src, engs)

                if ci == 0:
                    nc.scalar.copy(out=xt[:, :, 0:C],
                                   in_=xt[:, :, 4 * C:5 * C])
                    nc.scalar.copy(out=xt[:, :, C:2 * C],
                                   in_=xt[:, :, 3 * C:4 * C])
                if ci == N_CHUNK - 1:
                    wc = W_CHUNK
                    nc.scalar.copy(out=xt[:, :, (wc + 2) * C:(wc + 3) * C],
                                   in_=xt[:, :, wc * C:(wc + 1) * C])
                    nc.scalar.copy(out=xt[:, :, (wc + 3) * C:(wc + 4) * C],
                                   in_=xt[:, :, (wc - 1) * C:wc * C])
                return xt

            xt = emit_load(0)
            emit_p2(ctx, xt, tiles)
            for ci in range(N_CHUNK):
                w0 = ci * W_CHUNK
                emit_sort_rest(ctx, xt, tiles)
                if ci + 1 < N_CHUNK:
                    xt = emit_load(ci + 1)
                    emit_p2(ctx, xt, tiles)

                outt = iop.tile([128, R, WSEL], bf16, tag="outt", name="outt",
                                bufs=1)

                def post(outt=outt, w0=w0):
                    oengs = [nc.sync, nc.gpsimd]
                    for img in range(2):
                        p0 = img * NBLK
                        half = NBLK // 2
                        for hs in range(2):
                            dst = bass.AP(
                                y, img * IMG + hs * half * R * ROW + w0 * C,
                                [[R * ROW, half], [ROW, R], [1, WSEL]])
                            dma(dst,
                                outt[p0 + hs * half:p0 + (hs + 1) * half,
                                     :, :], oengs)

                emit_merge_sel(ctx, (outt[:], post), tiles)

    nc.finalize()
    return nc


# ---------------------------------------------------------------------------
# Numpy simulation of one core (for offline verification of the op list)
# ---------------------------------------------------------------------------

def simulate_core(x2):
    """x2: [2, H, W, C] float32 (pre-rounded to bf16 grid). Returns
    [2, H, W, C] median-filter output computed via the exact op list."""
    out = np.zeros_like(x2)
    xp = np.pad(x2, ((0, 0), (2, 2), (0, 0), (0, 0)), mode="reflect")
    for ci in range(N_CHUNK):
        w0 = ci * W_CHUNK
        pxlo = max(0, w0 - 2)
        pxhi = min(W, w0 + W_CHUNK + 2)
        n = (pxhi - pxlo) * C
        elo = (pxlo - (w0 - 2)) * C

        ctx = NumpyCtx()
        tiles = {k: ctx.tile(k, nsl) for k, nsl in
                 (("S", 5), ("SP", 5), ("LH", 10), ("PM", 10),
                  ("PMP", 10), ("Q", 4), ("FP", 4), ("X2", 2), ("X3", 2))}
        xt = np.full((128, 10, WS), np.nan, dtype=np.float32)
        for img in range(2):
            p0 = img * NBLK
            rows = xp[img].reshape(H + 4, ROW)
            for hb in range(NBLK):
                r0 = hb * R  # padded-row index of first input row
                xt[p0 + hb, :, elo:elo + n] = \
                    rows[r0:r0 + 10, pxlo * C:pxlo * C + n]
        if ci == 0:
            xt[:, :, 0:C] = xt[:, :, 4 * C:5 * C]
            xt[:, :, C:2 * C] = xt[:, :, 3 * C:4 * C]
        if ci == N_CHUNK - 1:
            wc = W_CHUNK
            xt[:, :, (wc + 2) * C:(wc + 3) * C] = xt[:, :, wc * C:(wc + 1) * C]
            xt[:, :, (wc + 3) * C:(wc + 4) * C] = \
                xt[:, :, (wc - 1) * C:wc * C]

        outt = np.full((128, R, WSEL), np.nan, dtype=np.float32)
        emit_p2(ctx, xt, tiles)
        emit_sort_rest(ctx, xt, tiles)
        emit_merge_sel(ctx, (outt, lambda: None), tiles)

        for img in range(2):
            p0 = img * NBLK
            o = outt[p0:p0 + NBLK].reshape(H, WSEL)
            out[img, :, w0:w0 + W_CHUNK, :] = o.reshape(H, W_CHUNK, C)
    return out


_NC = None


def _get_nc():
    global _NC
    if _NC is None:
        _NC = build_nc()
    return _NC


def kernel(x, k):
    assert int(k) == 5
    x = np.asarray(x, dtype=np.float32)
    assert x.shape == (16, H, W, C)
    xb = np.ascontiguousarray(x.astype(ml_dtypes.bfloat16))
    nc = _get_nc()
    in_maps = [{"x": xb[2 * i:2 * i + 2]} for i in range(8)]
    res = run_bass_kernel_spmd(nc, in_maps, core_ids=list(range(8)))
    out = np.concatenate([np.asarray(r["out"]) for r in res.results], axis=0)
    return out.astype(np.float32)


# revision 30
# speedup vs baseline: 1.0462x; 1.0014x over previous
"""Trainium2 Bass kernel for 5x5 median filter (reflect padding, SAME size).

Input x: [16, 384, 384, 3] f32 (NHWC), k=5. Output: same shape.

Strategy:
- Pure data parallel over 8 NeuronCores: 2 images per core.
- All compute in bf16: DVE tensor_tensor runs in 2x_1P perf mode for
  16-bit dtypes (vs 1x for f32). Median selection commutes with the
  monotone f32->bf16 rounding, so the result equals round_bf16(true
  median): rel err <= 2^-9. Host converts f32<->bf16 at the edges.
- Per core layout: partition p = img*64 + hblock, each hblock = 6 output
  rows. Free dim = (10 input rows) x (100 px * 3 ch) for a 96-px chunk
  (2 px halo each side). 4 chunks cover W=384.
- 2x_1P needs 4B-aligned operands; a 1-px shift is 3 bf16 els = 6B.
  All DVE ops use even element offsets {0, 6}; odd shifts are
  materialized as shifted copies (s' = s<<1px, pm' = pm<<1px) on the
  otherwise-idle Scalar engine, ordered so they hide under DVE work.
- Median-of-25 via separable sorting network, emitted as FUSED
  multi-plane DVE instructions (same-ALU ops over several planes in
  one instruction via slot-major tiles and strided 4-dim APs) to
  amortize the ~150-cycle per-instruction overhead:
  1. vertical sort of 5-row columns via sliding-window sharing:
     P2[i] = CE(rows i,i+1) fused over 9 rows; sorted5[r] =
     oddeven-merge(insert(row r+2 into P2[r]), P2[r+3])
  2. PM[x] = Batcher merge of sorted columns (x, x+1) -> sorted 10
  3. per window: L=PM[w-2], R=PM[w+1], M=sorted col w;
     u = mid-6 of merge(L,R) via DCE'd odd/even partial merges (the
     o/e partials are structurally identical -> fused 2-slot ops);
     median = rank 6 of merge(u, M).
- Reflect padding: row halos via DMAs from reflected rows, column halos
  via on-chip copies at image edges. DMAs round-robin over engine
  queues so chunk-0 issue latency shrinks.
"""

import numpy as np
import ml_dtypes

import concourse.bacc as bacc
import concourse.bass as bass
import concourse.mybir as mybir
from concourse.bass_utils import run_bass_kernel_spmd
from concourse.tile import TileContext

bf16 = mybir.dt.bfloat16
AMIN = mybir.AluOpType.min
AMAX = mybir.AluOpType.max

H = 384
W = 384
C = 3
ROW = W * C          # 1152 elements per image row
IMG = H * ROW        # elements per image
R = 6                # output rows per partition block
NBLK = H // R        # 64 blocks per image
W_CHUNK = 96         # output px per chunk
N_CHUNK = W // W_CHUNK

WS = (W_CHUNK + 4) * C    # column-sort domain width (els) = 300
WPM = 298                 # pair-merge op width (even, padded from 297)
WSEL = W_CHUNK * C        # selection/output domain width = 288
TW = 300                  # physical tile width for all work planes


# ---------------------------------------------------------------------------
# Emission context: bass backend + numpy simulation backend (for testing
# the op list without hardware).
# ---------------------------------------------------------------------------

class BassCtx:
    def __init__(self, nc, wp):
        self.nc = nc
        self.wp = wp
        self._tiles = {}

    def tile(self, tag, nslots):
        t = self.wp.tile([128, nslots * R, TW], bf16, tag=tag, name=tag)
        self._tiles[tag] = t
        return t

    # views ------------------------------------------------------------
    def sl(self, t, s0, ns=1, off=0, w=TW):
        """Contiguous slot range [s0, s0+ns), column window [off, off+w)."""
        return t[:, s0 * R:(s0 + ns) * R, off:off + w]

    def st(self, t, sl_, off=0, w=TW):
        """Strided slot view: sl_ is a python slice over slots."""
        r = t.rearrange("p (s r) w -> p s r w", r=R)
        return r[:, sl_, :, off:off + w]

    def xt_rows(self, xt, r0, nr, off=0, w=TW):
        return xt[:, r0:r0 + nr, off:off + w]

    # ops --------------------------------------------------------------
    def tt(self, op, out, in0, in1):
        self.nc.vector.tensor_tensor(out=out, in0=in0, in1=in1, op=op)

    def scopy(self, out, in_):
        self.nc.scalar.copy(out=out, in_=in_)

    def vcopy(self, out, in_):
        self.nc.vector.tensor_copy(out, in_)


class NumpyCtx:
    """Same op vocabulary over numpy arrays shaped [128, rows, TW]."""

    def __init__(self):
        self._tiles = {}

    def tile(self, tag, nslots):
        a = np.full((128, nslots * R, TW), np.nan, dtype=np.float32)
        self._tiles[tag] = a
        return a

    def sl(self, t, s0, ns=1, off=0, w=TW):
        return t[:, s0 * R:(s0 + ns) * R, off:off + w]

    def st(self, t, sl_, off=0, w=TW):
        r = t.reshape(128, -1, R, TW)
        return r[:, sl_, :, off:off + w]

    def xt_rows(self, xt, r0, nr, off=0, w=TW):
        return xt[:, r0:r0 + nr, off:off + w]

    def tt(self, op, out, in0, in1):
        f = np.minimum if op is AMIN else np.maximum
        res = f(in0.reshape(out.shape), in1.reshape(out.shape))
        out[...] = res

    def scopy(self, out, in_):
        out[...] = in_.reshape(out.shape)

    vcopy = scopy


# ---------------------------------------------------------------------------
# One chunk: sort -> merge -> selection, with fused DVE ops and
# ScalarE shifted copies.
# ---------------------------------------------------------------------------

def emit_p2(ctx, xt, tiles):
    """Shared sorted-pairs of vertically adjacent rows: P2[i] =
    CE(row i, row i+1) for i=0..8, as two fused 9-row ops. Each pair
    is reused by the sorted-triple above it and the tail pair below
    (sliding-window sharing). Emitted one chunk ahead so it fills the
    DVE stall while ScalarE finishes the previous chunk's s' copies."""
    X2, X3 = tiles["X2"], tiles["X3"]
    ctx.tt(AMIN, X2[:, 0:9, :], ctx.xt_rows(xt, 0, 9),
           ctx.xt_rows(xt, 1, 9))
    ctx.tt(AMAX, X3[:, 0:9, :], ctx.xt_rows(xt, 0, 9),
           ctx.xt_rows(xt, 1, 9))


def emit_sort_rest(ctx, xt, tiles):
    """sorted5[r] = merge(sorted3(rows r..r+2), pair(rows r+3, r+4))
    via odd-even merge(3,2). sorted3 = insert row r+2 into P2[r]."""
    S, SP, Q, FP, X2, X3 = (tiles[k] for k in
                            ("S", "SP", "Q", "FP", "X2", "X3"))
    MIN, MAX = AMIN, AMAX
    sl, st = ctx.sl, ctx.st
    x6 = ctx.xt_rows(xt, 2, R)
    p2lo, p2hi = X2[:, 0:R, :], X3[:, 0:R, :]
    b0, b1 = X2[:, 3:3 + R, :], X3[:, 3:3 + R, :]
    ctx.tt(MIN, sl(Q, 0), x6, p2lo)                   # a0
    ctx.tt(MAX, sl(Q, 1), x6, p2lo)                   # t
    ctx.tt(MIN, sl(Q, 2), sl(Q, 1), p2hi)             # a1
    ctx.tt(MAX, sl(Q, 3), sl(Q, 1), p2hi)             # a2
    ctx.tt(MIN, sl(S, 0), b0, sl(Q, 0))               # s0
    ctx.scopy(sl(SP, 0, 1, 0, WS - C), sl(S, 0, 1, C, WS - C))
    ctx.tt(MAX, sl(Q, 1), b0, sl(Q, 0))               # t' (odd-merge)
    ctx.tt(MIN, sl(FP, 0), sl(Q, 1), sl(Q, 3))        # z1
    ctx.tt(MAX, sl(FP, 1), sl(Q, 1), sl(Q, 3))        # z2
    ctx.tt(MIN, sl(FP, 2), sl(Q, 2), b1)              # d0
    ctx.tt(MAX, sl(FP, 3), sl(Q, 2), b1)              # d1
    ctx.tt(MIN, st(S, slice(1, 4, 2)), sl(FP, 2, 2), sl(FP, 0, 2))  # s1,s3
    ctx.tt(MAX, st(S, slice(2, 5, 2)), sl(FP, 2, 2), sl(FP, 0, 2))  # s2,s4
    # 4 of 5 sorted outputs land in the last two (fused) ops; put 3 of
    # the shifted copies on the DVE itself (fast 2x/4x tensor_copy) so
    # ScalarE's serial chain doesn't stall the merge
    ctx.scopy(sl(SP, 4, 1, 0, WS - C), sl(S, 4, 1, C, WS - C))
    ctx.vcopy(sl(SP, 1, 1, 0, WS - C), sl(S, 1, 1, C, WS - C))
    ctx.vcopy(sl(SP, 2, 1, 0, WS - C), sl(S, 2, 1, C, WS - C))
    ctx.vcopy(sl(SP, 3, 1, 0, WS - C), sl(S, 3, 1, C, WS - C))


def emit_merge_sel(ctx, outt_halves, tiles):
    S, SP, LH, PM, PMP, Q, FP = (tiles[k] for k in
                                 ("S", "SP", "LH", "PM", "PMP", "Q", "FP"))
    MIN, MAX = AMIN, AMAX
    sl, st = ctx.sl, ctx.st

    # ---- pair merge: PM[x] = merge(s[x], s[x+1]), b = s' ----
    # LH slots 0..4 = lo_i = min(a_i, b_i); slots 5..9 = hi_i.
    ctx.tt(MIN, sl(LH, 0, 5, 0, WPM), sl(S, 0, 5, 0, WPM),
           sl(SP, 0, 5, 0, WPM))
    ctx.tt(MAX, sl(LH, 5, 5, 0, WPM), sl(S, 0, 5, 0, WPM),
           sl(SP, 0, 5, 0, WPM))
    # pm0 = lo0, pm9 = hi4 (+ their shifted copies) on ScalarE
    ctx.scopy(sl(PM, 0, 1, 0, WPM), sl(LH, 0, 1, 0, WPM))
    ctx.scopy(sl(PMP, 0, 1, 0, WPM - C), sl(LH, 0, 1, C, WPM - C))
    ctx.scopy(sl(PM, 9, 1, 0, WPM), sl(LH, 9, 1, 0, WPM))
    ctx.scopy(sl(PMP, 9, 1, 0, WPM - C), sl(LH, 9, 1, C, WPM - C))
    # Q = (h1m, gm1, h2m, gm2): merges of (hi0,lo4) and (hi1,lo3)
    ctx.tt(MIN, sl(Q, 0, 2, 0, WPM), st(LH, slice(5, 7), 0, WPM),
           st(LH, slice(4, 2, -1), 0, WPM))
    ctx.tt(MAX, sl(Q, 2, 2, 0, WPM), st(LH, slice(5, 7), 0, WPM),
           st(LH, slice(4, 2, -1), 0, WPM))
    # FP = (f1, f2, f3, f4): f1,f2 = merge(lo2, h1m); f3,f4 = (hi2, h2m)
    ctx.tt(MIN, st(FP, slice(0, 3, 2), 0, WPM),
           st(LH, slice(2, 8, 5), 0, WPM), st(Q, slice(0, 3, 2), 0, WPM))
    ctx.tt(MAX, st(FP, slice(1, 4, 2), 0, WPM),
           st(LH, slice(2, 8, 5), 0, WPM), st(Q, slice(0, 3, 2), 0, WPM))
    # loop: pm_{2i+1} = min(g_i, f_{i+1}); pm_{2i+2} = max.
    # g = (lo1, gm1, gm2, hi3) = (LH1, Q1, Q3, LH8); f = FP0..FP3.
    # singles, ordered so the ScalarE pm' copies (same order) finish
    # before the selection needs them: {2,3} first, then {6,7},{4,5},
    # {0,1},{8,9} pairs.
    g_ops = [  # (pm_idx, op, g_view, f_view)
        (2, MAX, sl(LH, 1, 1, 0, WPM), sl(FP, 0, 1, 0, WPM)),
        (3, MIN, sl(Q, 1, 1, 0, WPM), sl(FP, 1, 1, 0, WPM)),
        (6, MAX, sl(Q, 3, 1, 0, WPM), sl(FP, 2, 1, 0, WPM)),
        (7, MIN, sl(LH, 8, 1, 0, WPM), sl(FP, 3, 1, 0, WPM)),
        (4, MAX, sl(Q, 1, 1, 0, WPM), sl(FP, 1, 1, 0, WPM)),
        (5, MIN, sl(Q, 3, 1, 0, WPM), sl(FP, 2, 1, 0, WPM)),
        (1, MIN, sl(LH, 1, 1, 0, WPM), sl(FP, 0, 1, 0, WPM)),
        (8, MAX, sl(LH, 8, 1, 0, WPM), sl(FP, 3, 1, 0, WPM)),
    ]
    for pi, op, gv, fv in g_ops:
        ctx.tt(op, sl(PM, pi, 1, 0, WPM), gv, fv)
        ctx.scopy(sl(PMP, pi, 1, 0, WPM - C), sl(PM, pi, 1, C, WPM - C))

    # ---- selection: mid-6 of merge(L,R), then rank 6 of merge(u, M) --
    # Fused 2-slot ops; sub-slot 0 = e-variant (even pm), 1 = o-variant.
    # Operand pairs: t1:{2,3} t2:{6,7} k:{4,5} t3:{0,1} t4:{8,9}.
    def pmpair(i):
        return (sl(PM, 2 * i, 2, 0, WSEL), sl(PMP, 2 * i, 2, 2 * C, WSEL))

    W2 = WSEL
    t1, t1b = sl(SP, 0, 2, 0, W2), pmpair(1)
    ctx.tt(MAX, t1, *t1b)
    t2 = sl(SP, 2, 2, 0, W2)
    ctx.tt(MIN, t2, *pmpair(3))
    g1 = sl(LH, 0, 2, 0, W2)
    ctx.tt(MIN, g1, t1, t2)
    g2 = sl(LH, 2, 2, 0, W2)
    ctx.tt(MAX, g2, t1, t2)
    k0 = sl(LH, 5, 2, 0, W2)
    ctx.tt(MIN, k0, *pmpair(2))
    k1 = sl(LH, 7, 2, 0, W2)
    ctx.tt(MAX, k1, *pmpair(2))
    t3 = sl(Q, 0, 2, 0, W2)
    ctx.tt(MAX, t3, *pmpair(0))
    t4 = sl(Q, 2, 2, 0, W2)
    ctx.tt(MIN, t4, *pmpair(4))
    h1 = sl(FP, 0, 2, 0, W2)
    ctx.tt(MIN, h1, t3, t4)
    h2 = sl(FP, 2, 2, 0, W2)
    ctx.tt(MAX, h2, t3, t4)
    f2 = sl(SP, 0, 2, 0, W2)      # overwrites t1 (dead)
    ctx.tt(MAX, f2, k0, h1)
    f3 = sl(SP, 2, 2, 0, W2)      # overwrites t2 (dead)
    ctx.tt(MIN, f3, k1, h2)

    # singles: o3,o4,o5 -> LH slots 5,6,7 ; e4,e5,e6 -> Q slots 0,1,2
    def sub(v2, j):  # sub-slot j of a 2-slot view
        return v2[:, j * R:(j + 1) * R, :]

    ctx.tt(MIN, sl(LH, 5, 1, 0, W2), sub(g1, 1), sub(f2, 1))   # o3
    ctx.tt(MAX, sl(LH, 6, 1, 0, W2), sub(g1, 1), sub(f2, 1))   # o4
    ctx.tt(MIN, sl(LH, 7, 1, 0, W2), sub(g2, 1), sub(f3, 1))   # o5
    ctx.tt(MAX, sl(Q, 0, 1, 0, W2), sub(g1, 0), sub(f2, 0))    # e4
    ctx.tt(MIN, sl(Q, 1, 1, 0, W2), sub(g2, 0), sub(f3, 0))    # e5
    ctx.tt(MAX, sl(Q, 2, 1, 0, W2), sub(g2, 0), sub(f3, 0))    # e6

    # u_i -> PM slots 0..5 (dead by now): u0,u2,u4 = min(o,e) strided
    ctx.tt(MIN, st(PM, slice(0, 6, 2), 0, W2), sl(LH, 5, 3, 0, W2),
           sl(Q, 0, 3, 0, W2))
    ctx.tt(MAX, st(PM, slice(1, 6, 2), 0, W2), sl(LH, 5, 3, 0, W2),
           sl(Q, 0, 3, 0, W2))

    # final: fused (x2,x1,k1p) = max(u_i, M_i) i=0..2 ;
    #        (q0,x3) = min(u_i, M_i) i=3..4
    ctx.tt(MAX, sl(FP, 0, 3, 0, W2), sl(PM, 0, 3, 0, W2),
           sl(S, 0, 3, 2 * C, W2))
    ctx.tt(MIN, sl(Q, 0, 2, 0, W2), sl(PM, 3, 2, 0, W2),
           sl(S, 3, 2, 2 * C, W2))
    ctx.tt(MIN, sl(LH, 0, 1, 0, W2), sl(FP, 1, 1, 0, W2),
           sl(PM, 5, 1, 0, W2))                                 # p1
    ctx.tt(MAX, sl(LH, 1, 1, 0, W2), sl(Q, 0, 1, 0, W2),
           sl(LH, 0, 1, 0, W2))                                 # o2p
    ctx.tt(MAX, sl(LH, 2, 1, 0, W2), sl(FP, 0, 1, 0, W2),
           sl(Q, 1, 1, 0, W2))                                  # h2p
    ctx.tt(MIN, sl(LH, 3, 1, 0, W2), sl(FP, 2, 1, 0, W2),
           sl(LH, 2, 1, 0, W2))                                 # e3p
    outv, post = outt_halves
    ctx.tt(MIN, outv, sl(LH, 1, 1, 0, W2), sl(LH, 3, 1, 0, W2))  # median
    post()


# ---------------------------------------------------------------------------
# Kernel builder
# ---------------------------------------------------------------------------

def build_nc():
    nc = bacc.Bacc("TRN2", target_bir_lowering=False)
    x = nc.dram_tensor("x", [2, H, W, C], bf16, kind="ExternalInput")
    y = nc.dram_tensor("out", [2, H, W, C], bf16, kind="ExternalOutput")
    dma_engs0 = [nc.sync, nc.gpsimd, nc.scalar]
    dma_engs = [nc.sync, nc.gpsimd]

    with TileContext(nc) as tc:
        with tc.tile_pool(name="io", bufs=2) as iop, \
             tc.tile_pool(name="work", bufs=1) as wp:
            ctx = BassCtx(nc, wp)
            tiles = {k: ctx.tile(k, n) for k, n in
                     (("S", 5), ("SP", 5), ("LH", 10), ("PM", 10),
                      ("PMP", 10), ("Q", 4), ("FP", 4), ("X2", 2),
                      ("X3", 2))}
            qi = 0

            def dma(out, in_, engs):
                nonlocal qi
                engs[qi % len(engs)].dma_start(out=out, in_=in_)
                qi += 1

            def emit_load(ci):
                w0 = ci * W_CHUNK
                pxlo = max(0, w0 - 2)
                pxhi = min(W, w0 + W_CHUNK + 2)
                n = (pxhi - pxlo) * C
                elo = (pxlo - (w0 - 2)) * C
                engs = dma_engs0 if ci == 0 else dma_engs
                xt = iop.tile([128, 10, WS], bf16, tag="xt", name="xt")
                for img in range(2):
                    base = img * IMG + pxlo * C
                    p0 = img * NBLK
                    # chunk 0 is latency-critical (nothing to overlap):
                    # split finer so transfers parallelize across rings
                    splits = [1, 33, 63]
                    for si in range(len(splits) - 1):
                        h0, h1 = splits[si], splits[si + 1]
                        src = bass.AP(x, base + (6 * h0 - 2) * ROW,
                                      [[6 * ROW, h1 - h0], [ROW, 10], [1, n]])
                        dma(xt[p0 + h0:p0 + h1, :, elo:elo + n], src, engs)
                    src = bass.AP(x, base, [[ROW, 1], [ROW, 8], [1, n]])
                    dma(xt[p0:p0 + 1, 2:10, elo:elo + n], src, engs)
                    for j, r in ((0, 2), (1, 1)):
                        src = bass.AP(x, base + r * ROW, [[ROW, 1], [1, n]])
                        dma(xt[p0:p0 + 1, j:j + 1, elo:elo + n], src, engs)
                    p63 = p0 + NBLK - 1
                    src = bass.AP(x, base + 376 * ROW,
                                  [[ROW, 1], [ROW, 8], [1, n]])
                    dma(xt[p63:p63 + 1, 0:8, elo:elo + n], src, engs)
                    for j, r in ((8, 382), (9, 381)):
                        src = bass.AP(x, base + r * ROW, [[ROW, 1], [1, n]])
                        dma(xt[p63:p63 + 1, j:j + 1, elo:elo + n], src, engs)

                if ci == 0:
                    nc.scalar.copy(out=xt[:, :, 0:C],
                                   in_=xt[:, :, 4 * C:5 * C])
                    nc.scalar.copy(out=xt[:, :, C:2 * C],
                                   in_=xt[:, :, 3 * C:4 * C])
                if ci == N_CHUNK - 1:
                    wc = W_CHUNK
                    nc.scalar.copy(out=xt[:, :, (wc + 2) * C:(wc + 3) * C],
                                   in_=xt[:, :, wc * C:(wc + 1) * C])
                    nc.scalar.copy(out=xt[:, :, (wc + 3) * C:(wc + 4) * C],
                                   in_=xt[:, :, (wc - 1) * C:wc * C])
                return xt

            xt = emit_load(0)
            emit_p2(ctx, xt, tiles)
            for ci in range(N_CHUNK):
                w0 = ci * W_CHUNK
                emit_sort_rest(ctx, xt, tiles)
                if ci + 1 < N_CHUNK:
                    xt = emit_load(ci + 1)
                    emit_p2(ctx, xt, tiles)

                outt = iop.tile([128, R, WSEL], bf16, tag="outt", name="outt",
                                bufs=1)

                def post(outt=outt, w0=w0):
                    oengs = [nc.sync, nc.gpsimd]
                    for img in range(2):
                        p0 = img * NBLK
                        half = NBLK // 2
                        for hs in range(2):
                            dst = bass.AP(
                                y, img * IMG + hs * half * R * ROW + w0 * C,
                                [[R * ROW, half], [ROW, R], [1, WSEL]])
                            dma(dst,
                                outt[p0 + hs * half:p0 + (hs + 1) * half,
                                     :, :], oengs)

                emit_merge_sel(ctx, (outt[:], post), tiles)

    nc.finalize()
    return nc


# ---------------------------------------------------------------------------
# Numpy simulation of one core (for offline verification of the op list)
# ---------------------------------------------------------------------------

def simulate_core(x2):
    """x2: [2, H, W, C] float32 (pre-rounded to bf16 grid). Returns
    [2, H, W, C] median-filter output computed via the exact op list."""
    out = np.zeros_like(x2)
    xp = np.pad(x2, ((0, 0), (2, 2), (0, 0), (0, 0)), mode="reflect")
    for ci in range(N_CHUNK):
        w0 = ci * W_CHUNK
        pxlo = max(0, w0 - 2)
        pxhi = min(W, w0 + W_CHUNK + 2)
        n = (pxhi - pxlo) * C
        elo = (pxlo - (w0 - 2)) * C

        ctx = NumpyCtx()
        tiles = {k: ctx.tile(k, nsl) for k, nsl in
                 (("S", 5), ("SP", 5), ("LH", 10), ("PM", 10),
                  ("PMP", 10), ("Q", 4), ("FP", 4), ("X2", 2), ("X3", 2))}
        xt = np.full((128, 10, WS), np.nan, dtype=np.float32)
        for img in range(2):
            p0 = img * NBLK
            rows = xp[img].reshape(H + 4, ROW)
            for hb in range(NBLK):
                r0 = hb * R  # padded-row index of first input row
                xt[p0 + hb, :, elo:elo + n] = \
                    rows[r0:r0 + 10, pxlo * C:pxlo * C + n]
        if ci == 0:
            xt[:, :, 0:C] = xt[:, :, 4 * C:5 * C]
            xt[:, :, C:2 * C] = xt[:, :, 3 * C:4 * C]
        if ci == N_CHUNK - 1:
            wc = W_CHUNK
            xt[:, :, (wc + 2) * C:(wc + 3) * C] = xt[:, :, wc * C:(wc + 1) * C]
            xt[:, :, (wc + 3) * C:(wc + 4) * C] = \
                xt[:, :, (wc - 1) * C:wc * C]

        outt = np.full((128, R, WSEL), np.nan, dtype=np.float32)
        emit_p2(ctx, xt, tiles)
        emit_sort_rest(ctx, xt, tiles)
        emit_merge_sel(ctx, (outt, lambda: None), tiles)

        for img in range(2):
            p0 = img * NBLK
            o = outt[p0:p0 + NBLK].reshape(H, WSEL)
            out[img, :, w0:w0 + W_CHUNK, :] = o.reshape(H, W_CHUNK, C)
    return out


_NC = None


def _get_nc():
    global _NC
    if _NC is None:
        _NC = build_nc()
    return _NC


def kernel(x, k):
    assert int(k) == 5
    x = np.asarray(x, dtype=np.float32)
    assert x.shape == (16, H, W, C)
    xb = np.ascontiguousarray(x.astype(ml_dtypes.bfloat16))
    nc = _get_nc()
    in_maps = [{"x": xb[2 * i:2 * i + 2]} for i in range(8)]
    res = run_bass_kernel_spmd(nc, in_maps, core_ids=list(range(8)))
    out = np.concatenate([np.asarray(r["out"]) for r in res.results], axis=0)
    return out.astype(np.float32)
